# revision 1
# baseline (speedup 1.0000x reference)
"""BartAttention (focused-attention variant) Trainium2 Bass kernel.

Problem (hardcoded): B=2, T=2048, D=1024, H=16 heads, hd=64.
  q = (h @ Wq.T + bq) * hd**-0.5 ; k = h @ Wk.T + bk ; v = h @ Wv.T + bv
  scores = q @ k.T per head ; e = f * exp(scores) ; attn = e / rowsum(e)
  out = (attn @ v) @ Wo.T + bo

Sharding over 8 cores: batch (2) x head-group (4 groups of 4 heads).
Each core computes its heads' QKV, attention, and a partial out-projection
(contraction over its 256 d-columns of Wo); host sums the 4 partials per
batch and adds bo.

On-device layout (per core):
  hT   [1024, 2048] bf16   hidden.T               (c on partitions)
  qT,kT [256, 2048] bf16   q.T / k.T              (head*hd on partitions)
  v    [2048, 4, 65] bf16  v per head + ones col  (s on partitions)
  scores computed transposed: sT[s,t] = k @ q.T so that e=f.T*exp(sT) has
  s on partitions, which is the contraction dim of the PV matmul.
  PV: outT_aug[65, t] = [v | 1].T @ e  -> row 64 = rowsum(e) per t (exact fp32).
  out-proj: final[t, m] = outT.T @ Wo_slice.T, scaled per head by 1/rowsum
  (broadcast via K=1 fp32r matmul before the multiply).

Schedule notes: QKV is emitted t-chunk-major following the hT DMA chunks, with
the first head-pair's attention interleaved so ACT (exp) starts early; the
per-head-pair PV accumulators use 2 PSUM banks, freed quickly by a deferred
normalization (recip + raw copy first, broadcast-multiply later).
"""

import numpy as np
import ml_dtypes

import concourse.bass as bass
import concourse.bacc as bacc
import concourse.mybir as mybir
from concourse.tile import TileContext
from concourse.bass_utils import run_bass_kernel_spmd

BF16 = mybir.dt.bfloat16
F32 = mybir.dt.float32
F32R = mybir.dt.float32r
AF = mybir.ActivationFunctionType

B, T, D = 2, 2048, 1024
H, HD = 16, 64
HG = 4               # heads per core
R = HG * HD          # 256 d-rows per core
SCALING = HD ** -0.5
N_CORES = 8

P = 128
KT = D // P          # 8 k-tiles for QKV contraction
MT = R // P          # 2 m-tiles of qT/kT rows
NCH = T // 512       # 4 chunks of 512 along t
ST = T // P          # 16 s-tiles

GPS_MOD = 0          # every GPS_MOD-th f-multiply goes to GPSIMD (0 = none)
E_BUFS = 6


def build_bass():
    nc = bacc.Bacc()

    hT_d = nc.declare_dram_parameter("hT", [D, T], BF16, isOutput=False)
    fT_d = nc.declare_dram_parameter("fT", [T, T], BF16, isOutput=False)
    wqT_d = nc.declare_dram_parameter("wqT", [D, R], BF16, isOutput=False)
    wkT_d = nc.declare_dram_parameter("wkT", [D, R], BF16, isOutput=False)
    wvT_d = nc.declare_dram_parameter("wvT", [D, R], BF16, isOutput=False)
    woT_d = nc.declare_dram_parameter("woT", [R, D], BF16, isOutput=False)
    bq_d = nc.declare_dram_parameter("bq", [R, 1], F32, isOutput=False)
    bk_d = nc.declare_dram_parameter("bk", [R, 1], F32, isOutput=False)
    bv_d = nc.declare_dram_parameter("bv", [1, R], BF16, isOutput=False)
    out_d = nc.declare_dram_parameter("out_partial", [T, D], F32, isOutput=True)

    with TileContext(nc) as tc:
        with (
            nc.allow_low_precision(reason="bf16/f32r pipeline is intentional"),
            tc.tile_pool(name="sb", bufs=1) as sb,
            tc.tile_pool(name="ps", bufs=1, space="PSUM") as ps,
        ):
            # ---- persistent SBUF tensors ----
            hT = sb.tile([P, KT, T], BF16)
            wqT = sb.tile([P, KT, R], BF16)
            wkT = sb.tile([P, KT, R], BF16)
            wvT = sb.tile([P, KT, R], BF16)
            woT = sb.tile([P, MT, D], BF16)
            bq = sb.tile([P, MT], F32)
            bk = sb.tile([P, MT], F32)
            bv = sb.tile([1, R], BF16)
            ones_r = sb.tile([1, P], BF16)     # K=1 lhsT for v-bias matmul
            ones64 = sb.tile([1, HD], F32R)    # K=1 lhsT for rowsum broadcast
            qT = sb.tile([P, MT, T], BF16)
            kT = sb.tile([P, MT, T], BF16)
            vsb = sb.tile([P, ST, HG, HD + 1], BF16)
            po = sb.tile([P, MT, T], BF16)     # scaled outT, out-proj lhsT
            eu01 = sb.tile([P, ST, 1024], BF16)  # unit (0,1) e, PV deferred

            nc.sync.dma_start(wqT[:], wqT_d.rearrange("(k p) r -> p k r", p=P))
            nc.sync.dma_start(wkT[:], wkT_d.rearrange("(k p) r -> p k r", p=P))
            nc.sync.dma_start(bq[:], bq_d.rearrange("(m p) one -> p (m one)", p=P))
            nc.sync.dma_start(bk[:], bk_d.rearrange("(m p) one -> p (m one)", p=P))
            nc.sync.dma_start(bv[:], bv_d[:])
            ones64_f32 = sb.tile([1, HD], F32)
            nc.vector.memset(ones_r[:], 1.0)
            nc.vector.memset(ones64_f32[:], 1.0)
            nc.vector.tensor_copy(ones64[:], ones64_f32[:])
            nc.vector.memset(vsb[:, :, :, HD : HD + 1], 1.0)

            hT_r = hT_d.rearrange("(k p) t -> p k t", p=P)
            mul_i = [0]

            def qkv_chunk(n):
                """QKV outputs for t-columns [n*512, (n+1)*512)."""
                nsl = slice(n * 512, (n + 1) * 512)
                for kk in range(0, KT, 2):
                    nc.sync.dma_start(
                        hT[:, kk : kk + 2, nsl], hT_r[:, kk : kk + 2, nsl]
                    )
                for w_sb, b_sb, o_sb in ((wqT, bq, qT), (wkT, bk, kT)):
                    for m in range(MT):
                        acc = ps.tile([P, 512], F32, tag="pv", bufs=4,
                                      name=f"qkacc_{n}_{m}")
                        for k in range(KT):
                            nc.tensor.matmul(
                                acc[:],
                                w_sb[:, k, m * P : (m + 1) * P],
                                hT[:, k, nsl],
                                start=(k == 0),
                                stop=(k == KT - 1),
                            )
                        nc.vector.tensor_scalar_add(
                            o_sb[:, m, nsl], acc[:], b_sb[:, m : m + 1]
                        )
                if n == 0:
                    nc.sync.dma_start(
                        wvT[:], wvT_d.rearrange("(k p) r -> p k r", p=P)
                    )
                for s in range(4 * n, 4 * n + 4):
                    acc = ps.tile([P, R], F32, tag="pv", bufs=4, name=f"vacc_{s}")
                    for k in range(KT):
                        nc.tensor.matmul(
                            acc[:],
                            hT[:, k, s * P : (s + 1) * P],
                            wvT[:, k, :],
                            start=(k == 0),
                            stop=False,
                        )
                    nc.tensor.matmul(acc[:], ones_r[:], bv[:], start=False, stop=True)
                    nc.vector.tensor_copy(
                        vsb[:, s, :, 0:HD],
                        acc[:].rearrange("p (h d) -> p h d", h=HG),
                    )

            def ft_load(tch, st):
                ftt = ft_tiles[tch]
                nc.sync.dma_start(
                    ftt[:, st, :],
                    fT_d[st * P : (st + 1) * P, tch * 512 : (tch + 1) * 512],
                )

            def attn_steps(tch, j, pvp, st_range):
                """Scores/exp/f-mul/PV for head pair j over st_range."""
                tsl = slice(tch * 512, (tch + 1) * 512)
                ftt = ft_tiles[tch]
                for st in st_range:
                    ssl = slice(st * P, (st + 1) * P)
                    sc = ps.tile([P, 1024], F32, tag="sc", bufs=2,
                                 name=f"sc_{tch}_{j}_{st}")
                    e = sb.tile([P, 1024], BF16, tag="e", bufs=E_BUFS,
                                name=f"e_{tch}_{j}_{st}")
                    for a in range(2):
                        rows = slice(a * HD, (a + 1) * HD)
                        nc.tensor.matmul(
                            sc[:, a * 512 : (a + 1) * 512],
                            kT[rows, j, ssl],
                            qT[rows, j, tsl],
                            start=True,
                            stop=True,
                        )
                    nc.scalar.activation(e[:], sc[:], AF.Exp)
                    for a in range(2):
                        half = slice(a * 512, (a + 1) * 512)
                        mul_i[0] += 1
                        use_gps = GPS_MOD and (mul_i[0] % GPS_MOD == 0)
                        eng = nc.gpsimd if use_gps else nc.vector
                        eng.tensor_mul(e[:, half], e[:, half], ftt[:, st, :])
                        nc.tensor.matmul(
                            pvp[a][:],
                            vsb[:, st, 2 * j + a, :],
                            e[:, half],
                            start=(st == 0),
                            stop=(st == ST - 1),
                        )

            pending_norms = []

            def norm_fast(tch, j, pvp):
                """Free the PSUM accumulators quickly: reciprocal of the
                rowsum row + raw copy-out; the scale multiply is deferred."""
                for a in range(2):
                    h = 2 * j + a
                    recip = sb.tile([1, 512], F32R, tag="recip", bufs=4,
                                    name=f"recip_{tch}_{h}")
                    praw = sb.tile([HD, 512], BF16, tag="praw", bufs=4,
                                   name=f"praw_{tch}_{h}")
                    nc.vector.reciprocal(recip[:], pvp[a][HD : HD + 1, :])
                    # praw on ACT: runs concurrently with the DVE recips, so
                    # the PSUM pair frees in half the time at pair boundaries
                    nc.scalar.copy(praw[:], pvp[a][0:HD, :])
                    pending_norms.append((tch, h, recip, praw))

            def norm_defer():
                """Broadcast 1/rowsum (K=1 f32r matmul) and scale into po."""
                while pending_norms:
                    tch, h, recip, praw = pending_norms.pop(0)
                    bcs = sb.tile([HD, 512], BF16, tag="bcs", bufs=4,
                                  name=f"bcs_{tch}_{h}")
                    bcp = ps.tile([HD, 512], F32, tag="sc", bufs=2,
                                  name=f"bcp_{tch}_{h}")
                    nc.tensor.matmul(bcp[:], ones64[:], recip[:], start=True, stop=True)
                    nc.vector.tensor_copy(bcs[:], bcp[:])
                    nc.vector.tensor_mul(
                        po[(h % 2) * HD : (h % 2) * HD + HD, h // 2,
                           tch * 512 : (tch + 1) * 512],
                        praw[:],
                        bcs[:],
                    )

            ft_tiles = {}

            def u01_scores(st_range):
                """Unit (0,1) scores/exp/f-mul during QKV; e kept in SBUF,
                PV deferred so no PSUM accumulators are held early."""
                for st in st_range:
                    ssl = slice(st * P, (st + 1) * P)
                    sc = ps.tile([P, 1024], F32, tag="sc", bufs=2,
                                 name=f"sc01_{st}")
                    for a in range(2):
                        rows = slice(a * HD, (a + 1) * HD)
                        nc.tensor.matmul(
                            sc[:, a * 512 : (a + 1) * 512],
                            kT[rows, 1, ssl],
                            qT[rows, 1, 0:512],
                            start=True,
                            stop=True,
                        )
                    nc.scalar.activation(eu01[:, st, :], sc[:], AF.Exp)
                    for a in range(2):
                        nc.vector.tensor_mul(
                            eu01[:, st, a * 512 : (a + 1) * 512],
                            eu01[:, st, a * 512 : (a + 1) * 512],
                            ft_tiles[0][:, st, :],
                        )

            def u01_pv_step(pvp, st):
                for a in range(2):
                    nc.tensor.matmul(
                        pvp[a][:],
                        vsb[:, st, 2 + a, :],
                        eu01[:, st, a * 512 : (a + 1) * 512],
                        start=(st == 0),
                        stop=(st == ST - 1),
                    )

            def outproj_unit(u):
                tt, n = divmod(u, 2)
                fin = ps.tile([P, 512], F32, tag="pv", bufs=4, name=f"fin_{tt}_{n}")
                osb = sb.tile([P, 512], F32, tag="osb", bufs=3, name=f"osb_{tt}_{n}")
                for j in range(MT):
                    nc.tensor.matmul(
                        fin[:],
                        po[:, j, tt * P : (tt + 1) * P],
                        woT[:, j, n * 512 : (n + 1) * 512],
                        start=(j == 0),
                        stop=(j == MT - 1),
                    )
                if tt % 2 == 0:
                    nc.scalar.copy(osb[:], fin[:])
                else:
                    nc.vector.tensor_copy(osb[:], fin[:])
                nc.sync.dma_start(
                    out_d[tt * P : (tt + 1) * P, n * 512 : (n + 1) * 512], osb[:]
                )

            def new_pv_pair(tch, j):
                return [ps.tile([HD + 1, 512], F32, tag="pv", bufs=4,
                                name=f"pv_{tch}_{j}_{a}") for a in range(2)]

            def new_ft(tch):
                ft_tiles[tch] = sb.tile([P, ST, 512], BF16,
                                        tag=f"ft{tch % 2}", bufs=1, name=f"ft_t{tch}")

            # ---- emission ----
            # u(0,0) rides along the QKV chunks; the remaining 7 (tch, j)
            # units run as overlapping pairs (two independent
            # scores->exp->mul->PV chains keep every engine fed); the last
            # unit interleaves with the out-projection of finished t-chunks.
            new_ft(0)
            pv00 = new_pv_pair(0, 0)
            for n in range(NCH):
                qkv_chunk(n)
                for st in range(4 * n, 4 * n + 4):
                    ft_load(0, st)
                attn_steps(0, 0, pv00, range(4 * n, 4 * n + 4))
                u01_scores(range(4 * n, 4 * n + 4))
            norm_fast(0, 0, pv00)
            nc.sync.dma_start(woT[:], woT_d.rearrange("(m p) d -> p m d", p=P))

            pv01, pv10 = new_pv_pair(0, 1), new_pv_pair(1, 0)
            new_ft(1)
            for st in range(ST):
                ft_load(1, st)
            for st in range(ST):
                u01_pv_step(pv01, st)
                attn_steps(1, 0, pv10, (st,))
                if st == 2:
                    norm_defer()
            norm_fast(0, 1, pv01)
            norm_fast(1, 0, pv10)

            for ua, ub in (((1, 1), (2, 0)), ((2, 1), (3, 0))):
                for tch in (ua[0], ub[0]):
                    if tch not in ft_tiles:
                        new_ft(tch)
                        for st in range(ST):
                            ft_load(tch, st)
                pva, pvb = new_pv_pair(*ua), new_pv_pair(*ub)
                for st in range(ST):
                    attn_steps(*ua, pva, (st,))
                    attn_steps(*ub, pvb, (st,))
                    if st == 2:
                        norm_defer()
                norm_fast(*ua, pva)
                norm_fast(*ub, pvb)

            pv31 = new_pv_pair(3, 1)
            for st in range(ST):
                attn_steps(3, 1, pv31, (st,))
                if st == 2:
                    norm_defer()
                if st < 12:
                    outproj_unit(2 * st)
                    outproj_unit(2 * st + 1)
            norm_fast(3, 1, pv31)
            norm_defer()
            for u in range(24, 32):
                outproj_unit(u)

    return nc


_NC = None
_LAST_RESULT = None


def _get_nc():
    global _NC
    if _NC is None:
        _NC = build_bass()
        if not _NC.is_finalized():
            _NC.finalize()
    return _NC


def kernel(hidden_states, focused_attention, Wq, bq, Wk, bk, Wv, bv, Wo, bo):
    bf = ml_dtypes.bfloat16
    hT = [np.ascontiguousarray(hidden_states[b].T).astype(bf) for b in range(B)]
    fT = [np.ascontiguousarray(focused_attention[b].T).astype(bf) for b in range(B)]

    in_maps = []
    for c in range(N_CORES):
        b, g = divmod(c, 4)
        rows = slice(g * R, (g + 1) * R)
        in_maps.append({
            "hT": hT[b],
            "fT": fT[b],
            "wqT": np.ascontiguousarray((Wq[rows] * SCALING).T).astype(bf),
            "wkT": np.ascontiguousarray(Wk[rows].T).astype(bf),
            "wvT": np.ascontiguousarray(Wv[rows].T).astype(bf),
            "woT": np.ascontiguousarray(Wo[:, rows].T).astype(bf),
            "bq": np.ascontiguousarray((bq[rows] * SCALING)[:, None]).astype(np.float32),
            "bk": np.ascontiguousarray(bk[rows][:, None]).astype(np.float32),
            "bv": np.ascontiguousarray(bv[rows][None, :]).astype(bf),
        })

    res = run_bass_kernel_spmd(_get_nc(), in_maps, list(range(N_CORES)))
    global _LAST_RESULT
    _LAST_RESULT = res
    out = np.zeros((B, T, D), dtype=np.float32)
    for c in range(N_CORES):
        out[c // 4] += res.results[c]["out_partial"]
    out += np.asarray(bo, dtype=np.float32)[None, None, :]
    return out



# revision 7
# speedup vs baseline: 1.0459x; 1.0459x over previous
"""BartAttention (focused-attention variant) Trainium2 Bass kernel.

Problem (hardcoded): B=2, T=2048, D=1024, H=16 heads, hd=64.
  q = (h @ Wq.T + bq) * hd**-0.5 ; k = h @ Wk.T + bk ; v = h @ Wv.T + bv
  scores = q @ k.T per head ; e = f * exp(scores) ; attn = e / rowsum(e)
  out = (attn @ v) @ Wo.T + bo

Sharding over 8 cores: batch (2) x head-group (4 groups of 4 heads).
Each core computes its heads' QKV, attention, and a partial out-projection
(contraction over its 256 d-columns of Wo); host sums the 4 partials per
batch and adds bo.

On-device layout (per core):
  hT   [1024, 2048] bf16   hidden.T               (c on partitions)
  qT,kT [256, 2048] bf16   q.T / k.T              (head*hd on partitions)
  v    [2048, 4, 65] bf16  v per head + ones col  (s on partitions)
  scores computed transposed: sT[s,t] = k @ q.T so that e=f.T*exp(sT) has
  s on partitions, which is the contraction dim of the PV matmul.
  PV: outT_aug[65, t] = [v | 1].T @ e  -> row 64 = rowsum(e) per t (exact fp32).
  out-proj: final[t, m] = outT.T @ Wo_slice.T, scaled per head by 1/rowsum.

Engine placement (engine-balance: PE ~163us is the binding roofline; ACT
carries only the exps ~134us; DVE ~132us; Pool picks up all PSUM->SBUF
copies, the rowsum-reciprocal partition-broadcast, and out staging):
  PE:   QKV/scores/PV/out-proj matmuls only (plus a tiny warmup matmul that
        starts the p-state ramp clock before the first DMA lands).
  ACT:  exp activations only (plus the wq/wk weight-load DMA queue at start).
  DVE:  q/k bias adds, v copy-out, f-multiplies, reciprocal, po scale-mul.
  Pool: praw PSUM->SBUF copies, 1/rowsum partition-broadcast, osb staging.

Schedule: QKV is emitted t-chunk-major following the hT DMA chunks, with
head-pair (0,0) attention and the (0,1) scores (PV deferred) interleaved so
ACT starts early; the remaining units run as overlapping pairs; out-proj
units burst at pair boundaries (staggering the pair's second unit by 4
s-tiles so the fin PSUM tiles fit in the freed accumulator banks).
"""

import numpy as np
import ml_dtypes

import concourse.bass as bass
import concourse.bacc as bacc
import concourse.mybir as mybir
from concourse.tile import TileContext
from concourse.bass_utils import run_bass_kernel_spmd

BF16 = mybir.dt.bfloat16
F32 = mybir.dt.float32
AF = mybir.ActivationFunctionType

B, T, D = 2, 2048, 1024
H, HD = 16, 64
HG = 4               # heads per core
R = HG * HD          # 256 d-rows per core
SCALING = HD ** -0.5
N_CORES = 8

P = 128
KT = D // P          # 8 k-tiles for QKV contraction
MT = R // P          # 2 m-tiles of qT/kT rows
NCH = T // 512       # 4 chunks of 512 along t
ST = T // P          # 16 s-tiles

E_BUFS = 6


def build_bass():
    nc = bacc.Bacc()

    hT_d = nc.declare_dram_parameter("hT", [D, T], BF16, isOutput=False)
    fT_d = nc.declare_dram_parameter("fT", [T, T], BF16, isOutput=False)
    wqT_d = nc.declare_dram_parameter("wqT", [D, R], BF16, isOutput=False)
    wkT_d = nc.declare_dram_parameter("wkT", [D, R], BF16, isOutput=False)
    wvT_d = nc.declare_dram_parameter("wvT", [D, R], BF16, isOutput=False)
    woT_d = nc.declare_dram_parameter("woT", [R, D], BF16, isOutput=False)
    bq_d = nc.declare_dram_parameter("bq", [R, 1], F32, isOutput=False)
    bk_d = nc.declare_dram_parameter("bk", [R, 1], F32, isOutput=False)
    bv_d = nc.declare_dram_parameter("bv", [1, R], BF16, isOutput=False)
    out_d = nc.declare_dram_parameter("out_partial", [T, D], F32, isOutput=True)

    with TileContext(nc) as tc:
        with (
            nc.allow_low_precision(reason="bf16 pipeline is intentional"),
            tc.tile_pool(name="sb", bufs=1) as sb,
            tc.tile_pool(name="ps", bufs=1, space="PSUM") as ps,
        ):
            # ---- persistent SBUF tensors ----
            hT = sb.tile([P, KT, T], BF16)
            wqT = sb.tile([P, KT, R], BF16)
            wkT = sb.tile([P, KT, R], BF16)
            wvT = sb.tile([P, KT, R], BF16)
            woT = sb.tile([P, MT, D], BF16)
            bq = sb.tile([P, MT], F32)
            bk = sb.tile([P, MT], F32)
            bv = sb.tile([1, R], BF16)
            ones_r = sb.tile([1, P], BF16)     # K=1 lhsT for v-bias matmul
            qT = sb.tile([P, MT, T], BF16)
            kT = sb.tile([P, MT, T], BF16)
            vsb = sb.tile([P, ST, HG, HD + 1], BF16)
            po = sb.tile([P, MT, T], BF16)     # scaled outT, out-proj lhsT
            eu01 = sb.tile([P, ST, 1024], BF16)  # unit (0,1) e, PV deferred

            # warmup: a 1-column matmul as soon as ones_r is set starts the
            # PE p-state clock, so real matmuls (arriving ~4us later, past
            # the 3us ramp) run at full clock from the first chunk.
            nc.vector.memset(ones_r[:], 1.0)
            warm = ps.tile([1, 1], F32, tag="pv", bufs=4, name="warm")
            nc.tensor.matmul(warm[:], ones_r[:, 0:1], ones_r[:, 0:1],
                             start=True, stop=True)

            # startup DMAs: first hT chunk on the SP queue, q/k weights in
            # parallel on the ACT HWDGE queue (ACT is idle until scores
            # exist), so the first QKV matmul isn't serialized behind them.
            hT_r = hT_d.rearrange("(k p) t -> p k t", p=P)
            for kk in range(0, KT, 2):
                nc.sync.dma_start(hT[:, kk : kk + 2, 0:512],
                                  hT_r[:, kk : kk + 2, 0:512])
            nc.scalar.dma_start(wqT[:], wqT_d.rearrange("(k p) r -> p k r", p=P))
            nc.scalar.dma_start(wkT[:], wkT_d.rearrange("(k p) r -> p k r", p=P))
            nc.sync.dma_start(bq[:], bq_d.rearrange("(m p) one -> p (m one)", p=P))
            nc.sync.dma_start(bk[:], bk_d.rearrange("(m p) one -> p (m one)", p=P))
            nc.sync.dma_start(bv[:], bv_d[:])
            nc.vector.memset(vsb[:, :, :, HD : HD + 1], 1.0)

            def qkv_chunk(n):
                """QKV outputs for t-columns [n*512, (n+1)*512)."""
                nsl = slice(n * 512, (n + 1) * 512)
                if n > 0:
                    for kk in range(0, KT, 2):
                        nc.sync.dma_start(
                            hT[:, kk : kk + 2, nsl], hT_r[:, kk : kk + 2, nsl]
                        )
                for w_sb, b_sb, o_sb in ((wqT, bq, qT), (wkT, bk, kT)):
                    for m in range(MT):
                        acc = ps.tile([P, 512], F32, tag="pv", bufs=4,
                                      name=f"qkacc_{n}_{m}")
                        for k in range(KT):
                            nc.tensor.matmul(
                                acc[:],
                                w_sb[:, k, m * P : (m + 1) * P],
                                hT[:, k, nsl],
                                start=(k == 0),
                                stop=(k == KT - 1),
                            )
                        nc.vector.tensor_scalar_add(
                            o_sb[:, m, nsl], acc[:], b_sb[:, m : m + 1]
                        )
                if n == 0:
                    nc.sync.dma_start(
                        wvT[:], wvT_d.rearrange("(k p) r -> p k r", p=P)
                    )
                for s in range(4 * n, 4 * n + 4):
                    acc = ps.tile([P, R], F32, tag="pv", bufs=4, name=f"vacc_{s}")
                    for k in range(KT):
                        nc.tensor.matmul(
                            acc[:],
                            hT[:, k, s * P : (s + 1) * P],
                            wvT[:, k, :],
                            start=(k == 0),
                            stop=False,
                        )
                    nc.tensor.matmul(acc[:], ones_r[:], bv[:], start=False, stop=True)
                    nc.vector.tensor_copy(
                        vsb[:, s, :, 0:HD],
                        acc[:].rearrange("p (h d) -> p h d", h=HG),
                    )

            def ft_load(tch, st):
                ftt = ft_tiles[tch]
                nc.sync.dma_start(
                    ftt[:, st, :],
                    fT_d[st * P : (st + 1) * P, tch * 512 : (tch + 1) * 512],
                )

            def attn_steps(tch, j, pvp, st_range):
                """Scores/exp/f-mul/PV for head pair j over st_range."""
                tsl = slice(tch * 512, (tch + 1) * 512)
                ftt = ft_tiles[tch]
                for st in st_range:
                    ssl = slice(st * P, (st + 1) * P)
                    sc = ps.tile([P, 1024], F32, tag="sc", bufs=2,
                                 name=f"sc_{tch}_{j}_{st}")
                    e = sb.tile([P, 1024], BF16, tag="e", bufs=E_BUFS,
                                name=f"e_{tch}_{j}_{st}")
                    for a in range(2):
                        rows = slice(a * HD, (a + 1) * HD)
                        nc.tensor.matmul(
                            sc[:, a * 512 : (a + 1) * 512],
                            kT[rows, j, ssl],
                            qT[rows, j, tsl],
                            start=True,
                            stop=True,
                        )
                    nc.scalar.activation(e[:], sc[:], AF.Exp)
                    for a in range(2):
                        half = slice(a * 512, (a + 1) * 512)
                        nc.vector.tensor_mul(e[:, half], e[:, half], ftt[:, st, :])
                        nc.tensor.matmul(
                            pvp[a][:],
                            vsb[:, st, 2 * j + a, :],
                            e[:, half],
                            start=(st == 0),
                            stop=(st == ST - 1),
                        )

            pending_norms = []

            def norm_fast(tch, j, pvp):
                """Free the PSUM accumulators quickly: praw copy on Pool and
                the rowsum reciprocal on DVE; the scale multiply is deferred."""
                for a in range(2):
                    h = 2 * j + a
                    recip = sb.tile([1, 512], BF16, tag="recip", bufs=4,
                                    name=f"recip_{tch}_{h}")
                    praw = sb.tile([HD, 512], BF16, tag="praw", bufs=4,
                                   name=f"praw_{tch}_{h}")
                    nc.vector.reciprocal(recip[:], pvp[a][HD : HD + 1, :])
                    # GPSIMD has no PSUM access; split the copies ACT/DVE
                    if a == 0:
                        nc.scalar.copy(praw[:], pvp[a][0:HD, :])
                    else:
                        nc.vector.tensor_copy(praw[:], pvp[a][0:HD, :])
                    pending_norms.append((tch, h, recip, praw))

            def norm_defer():
                """Broadcast 1/rowsum across partitions (Pool) and scale."""
                while pending_norms:
                    tch, h, recip, praw = pending_norms.pop(0)
                    bcs = sb.tile([HD, 512], BF16, tag="bcs", bufs=4,
                                  name=f"bcs_{tch}_{h}")
                    nc.gpsimd.partition_broadcast(bcs[:], recip[:])
                    nc.vector.tensor_mul(
                        po[(h % 2) * HD : (h % 2) * HD + HD, h // 2,
                           tch * 512 : (tch + 1) * 512],
                        praw[:],
                        bcs[:],
                    )

            ft_tiles = {}

            def u01_scores(st_range):
                """Unit (0,1) scores/exp/f-mul during QKV; e kept in SBUF,
                PV deferred so no PSUM accumulators are held early."""
                for st in st_range:
                    ssl = slice(st * P, (st + 1) * P)
                    sc = ps.tile([P, 1024], F32, tag="sc", bufs=2,
                                 name=f"sc01_{st}")
                    for a in range(2):
                        rows = slice(a * HD, (a + 1) * HD)
                        nc.tensor.matmul(
                            sc[:, a * 512 : (a + 1) * 512],
                            kT[rows, 1, ssl],
                            qT[rows, 1, 0:512],
                            start=True,
                            stop=True,
                        )
                    nc.scalar.activation(eu01[:, st, :], sc[:], AF.Exp)
                    for a in range(2):
                        nc.vector.tensor_mul(
                            eu01[:, st, a * 512 : (a + 1) * 512],
                            eu01[:, st, a * 512 : (a + 1) * 512],
                            ft_tiles[0][:, st, :],
                        )

            def u01_pv_step(pvp, st):
                for a in range(2):
                    nc.tensor.matmul(
                        pvp[a][:],
                        vsb[:, st, 2 + a, :],
                        eu01[:, st, a * 512 : (a + 1) * 512],
                        start=(st == 0),
                        stop=(st == ST - 1),
                    )

            def outproj_unit(u):
                tt, n = divmod(u, 2)
                fin = ps.tile([P, 512], F32, tag="pv", bufs=4, name=f"fin_{tt}_{n}")
                osb = sb.tile([P, 512], F32, tag="osb", bufs=3, name=f"osb_{tt}_{n}")
                for j in range(MT):
                    nc.tensor.matmul(
                        fin[:],
                        po[:, j, tt * P : (tt + 1) * P],
                        woT[:, j, n * 512 : (n + 1) * 512],
                        start=(j == 0),
                        stop=(j == MT - 1),
                    )
                if tt % 2 == 0:
                    nc.scalar.copy(osb[:], fin[:])
                else:
                    nc.vector.tensor_copy(osb[:], fin[:])
                nc.sync.dma_start(
                    out_d[tt * P : (tt + 1) * P, n * 512 : (n + 1) * 512], osb[:]
                )

            def new_pv_pair(tch, j):
                return [ps.tile([HD + 1, 512], F32, tag="pv", bufs=4,
                                name=f"pv_{tch}_{j}_{a}") for a in range(2)]

            def new_ft(tch):
                ft_tiles[tch] = sb.tile([P, ST, 512], BF16,
                                        tag=f"ft{tch % 2}", bufs=1, name=f"ft_t{tch}")

            # ---- emission ----
            # u(0,0) and u(0,1)-scores ride along the QKV chunks (PE binds
            # this phase, ACT has slack); the remaining units run as
            # overlapping pairs; out-proj units burst at pair boundaries,
            # interleaved into the first 4 s-tiles while only one new PV
            # pair holds PSUM banks.
            new_ft(0)
            pv00 = new_pv_pair(0, 0)
            for n in range(NCH):
                qkv_chunk(n)
                for st in range(4 * n, 4 * n + 4):
                    ft_load(0, st)
                attn_steps(0, 0, pv00, range(4 * n, 4 * n + 4))
                u01_scores(range(4 * n, 4 * n + 4))
            norm_fast(0, 0, pv00)
            nc.sync.dma_start(woT[:], woT_d.rearrange("(m p) d -> p m d", p=P))

            # P1: replay (0,1) PV from stashed e, run (1,0).
            pv01, pv10 = new_pv_pair(0, 1), new_pv_pair(1, 0)
            new_ft(1)
            for st in range(ST):
                ft_load(1, st)
            for st in range(ST):
                u01_pv_step(pv01, st)
                attn_steps(1, 0, pv10, (st,))
                if st == 2:
                    norm_defer()
            norm_fast(0, 1, pv01)
            norm_fast(1, 0, pv10)

            # P2/P3: unit pairs with a 4-st stagger; the 8 out-proj units of
            # the previously completed t-chunk fill the stagger window using
            # the PSUM banks the previous pair just freed.
            for (ua, ub), burst_tch in ((((1, 1), (2, 0)), 0),
                                        (((2, 1), (3, 0)), 1)):
                for tch in (ua[0], ub[0]):
                    if tch not in ft_tiles:
                        new_ft(tch)
                        for st in range(ST):
                            ft_load(tch, st)
                norm_defer()  # completes po for burst_tch before its reads
                pva, pvb = new_pv_pair(*ua), new_pv_pair(*ub)
                for st in range(4):
                    attn_steps(*ua, pva, (st,))
                    outproj_unit(8 * burst_tch + 2 * st)
                    outproj_unit(8 * burst_tch + 2 * st + 1)
                for st in range(4, ST):
                    attn_steps(*ua, pva, (st,))
                    attn_steps(*ub, pvb, (st - 4,))
                norm_fast(*ua, pva)
                for st in range(ST - 4, ST):
                    attn_steps(*ub, pvb, (st,))
                norm_fast(*ub, pvb)

            # P4: last unit (3,1) + t-chunk 2 out-proj burst.
            norm_defer()
            pv31 = new_pv_pair(3, 1)
            for st in range(ST):
                attn_steps(3, 1, pv31, (st,))
                if st < 8:
                    outproj_unit(16 + st)
            norm_fast(3, 1, pv31)
            norm_defer()
            for u in range(24, 32):
                outproj_unit(u)

    return nc


_NC = None
_LAST_RESULT = None


def _get_nc():
    global _NC
    if _NC is None:
        _NC = build_bass()
        if not _NC.is_finalized():
            _NC.finalize()
    return _NC


def kernel(hidden_states, focused_attention, Wq, bq, Wk, bk, Wv, bv, Wo, bo):
    bf = ml_dtypes.bfloat16
    hT = [np.ascontiguousarray(hidden_states[b].T).astype(bf) for b in range(B)]
    fT = [np.ascontiguousarray(focused_attention[b].T).astype(bf) for b in range(B)]

    in_maps = []
    for c in range(N_CORES):
        b, g = divmod(c, 4)
        rows = slice(g * R, (g + 1) * R)
        in_maps.append({
            "hT": hT[b],
            "fT": fT[b],
            "wqT": np.ascontiguousarray((Wq[rows] * SCALING).T).astype(bf),
            "wkT": np.ascontiguousarray(Wk[rows].T).astype(bf),
            "wvT": np.ascontiguousarray(Wv[rows].T).astype(bf),
            "woT": np.ascontiguousarray(Wo[:, rows].T).astype(bf),
            "bq": np.ascontiguousarray((bq[rows] * SCALING)[:, None]).astype(np.float32),
            "bk": np.ascontiguousarray(bk[rows][:, None]).astype(np.float32),
            "bv": np.ascontiguousarray(bv[rows][None, :]).astype(bf),
        })

    res = run_bass_kernel_spmd(_get_nc(), in_maps, list(range(N_CORES)))
    global _LAST_RESULT
    _LAST_RESULT = res
    out = np.zeros((B, T, D), dtype=np.float32)
    for c in range(N_CORES):
        out[c // 4] += res.results[c]["out_partial"]
    out += np.asarray(bo, dtype=np.float32)[None, None, :]
    return out


# revision 8
# speedup vs baseline: 1.1123x; 1.0635x over previous
"""BartAttention (focused-attention variant) Trainium2 Bass kernel.

Problem (hardcoded): B=2, T=2048, D=1024, H=16 heads, hd=64.
  q = (h @ Wq.T + bq) * hd**-0.5 ; k = h @ Wk.T + bk ; v = h @ Wv.T + bv
  scores = q @ k.T per head ; e = f * exp(scores) ; attn = e / rowsum(e)
  out = (attn @ v) @ Wo.T + bo

Sharding over 8 cores: batch (2) x head-group (4 groups of 4 heads).
Each core computes its heads' QKV, attention, and a partial out-projection
(contraction over its 256 d-columns of Wo); host sums the 4 bf16 partials
per batch in f32 and adds bo.

On-device layout (per core):
  hT   [1024, 2048] bf16   hidden.T               (c on partitions)
  qT,kT [256, 2048] bf16   q.T / k.T              (head*hd on partitions)
  v    [2048, 4, 65] bf16  v per head + ones col  (s on partitions)
  scores computed transposed: sT[s,t] = k @ q.T so that e=f.T*exp(sT) has
  s on partitions, which is the contraction dim of the PV matmul.
  PV: outT_aug[65, t] = [v | 1].T @ e  -> row 64 = rowsum(e) per t (exact fp32).
  out-proj: final[t, m] = outT.T @ Wo_slice.T, scaled per head by 1/rowsum
  (1/rowsum broadcast across partitions by GPSIMD, multiply on DVE at 2x).

Software pipeline: the 8 head-pair units (tch, j) are split into a "front"
(scores -> exp -> f-mul, needs only the 2 `sc` PSUM tiles, result stashed
in one of two SBUF e-stashes) and a "back" (16 PV accumulation steps from
the stash, needs only 2 `pv` PSUM banks).  Unit u0 runs live inside the
QKV chunk loop; fronts u1, u2 also run there (ACT has ~20us of slack
during QKV), then segments k=1..7 each run back(k) + front(k+2) + a few
out-projection burst units, so PE always has ACT-independent filler work
and the exp stream never gates the PSUM accumulators.

Engine placement: PE matmuls ~165us (binding); ACT = exps + q/k bias adds
+ late out-proj staging; DVE = f-muls, v copy, reciprocal, praw copies,
po scale-muls, early out-proj staging; Pool = 1/rowsum partition-broadcast.
"""

import numpy as np
import ml_dtypes

import concourse.bass as bass
import concourse.bacc as bacc
import concourse.mybir as mybir
from concourse.tile import TileContext
from concourse.bass_utils import run_bass_kernel_spmd

BF16 = mybir.dt.bfloat16
F32 = mybir.dt.float32
AF = mybir.ActivationFunctionType

B, T, D = 2, 2048, 1024
H, HD = 16, 64
HG = 4               # heads per core
R = HG * HD          # 256 d-rows per core
SCALING = HD ** -0.5
N_CORES = 8

P = 128
KT = D // P          # 8 k-tiles for QKV contraction
MT = R // P          # 2 m-tiles of qT/kT rows
NCH = T // 512       # 4 chunks of 512 along t
ST = T // P          # 16 s-tiles

E_BUFS = 5


def build_bass():
    nc = bacc.Bacc()

    hT_d = nc.declare_dram_parameter("hT", [D, T], BF16, isOutput=False)
    fT_d = nc.declare_dram_parameter("fT", [T, T], BF16, isOutput=False)
    wqT_d = nc.declare_dram_parameter("wqT", [D, R], BF16, isOutput=False)
    wkT_d = nc.declare_dram_parameter("wkT", [D, R], BF16, isOutput=False)
    wvT_d = nc.declare_dram_parameter("wvT", [D, R], BF16, isOutput=False)
    woT_d = nc.declare_dram_parameter("woT", [R, D], BF16, isOutput=False)
    bq_d = nc.declare_dram_parameter("bq", [R, 1], F32, isOutput=False)
    bk_d = nc.declare_dram_parameter("bk", [R, 1], F32, isOutput=False)
    bv_d = nc.declare_dram_parameter("bv", [1, R], BF16, isOutput=False)
    out_d = nc.declare_dram_parameter("out_partial", [T, D], BF16, isOutput=True)

    with TileContext(nc) as tc:
        with (
            nc.allow_low_precision(reason="bf16 pipeline is intentional"),
            tc.tile_pool(name="sb", bufs=1) as sb,
            tc.tile_pool(name="ps", bufs=1, space="PSUM") as ps,
        ):
            # ---- persistent SBUF tensors ----
            hT = sb.tile([P, KT, T], BF16)
            wqT = sb.tile([P, KT, R], BF16)
            wkT = sb.tile([P, KT, R], BF16)
            wvT = sb.tile([P, KT, R], BF16)
            woT = sb.tile([P, MT, D], BF16)
            bq = sb.tile([P, MT], F32)
            bk = sb.tile([P, MT], F32)
            bv = sb.tile([1, R], BF16)
            ones_r = sb.tile([1, P], BF16)     # K=1 lhsT for v-bias matmul
            qT = sb.tile([P, MT, T], BF16)
            kT = sb.tile([P, MT, T], BF16)
            vsb = sb.tile([P, ST, HG, HD + 1], BF16)
            po = sb.tile([P, MT, T], BF16)     # scaled outT, out-proj lhsT
            stash = [sb.tile([P, ST, 1024], BF16, name=f"stash{i}")
                     for i in range(2)]

            # warmup: a 1-column matmul as soon as ones_r is set starts the
            # PE p-state clock, so real matmuls (arriving ~4us later, past
            # the 3us ramp) run at full clock from the first chunk.
            nc.vector.memset(ones_r[:], 1.0)
            warm = ps.tile([1, 1], F32, tag="pv", bufs=4, name="warm")
            nc.tensor.matmul(warm[:], ones_r[:, 0:1], ones_r[:, 0:1],
                             start=True, stop=True)

            # startup DMAs: first hT chunk on the SP queue; q/k weights in
            # k-halves on the ACT HWDGE queue (ACT is idle until scores
            # exist), interleaved so the first QKV matmuls get their
            # operands as early as possible.
            hT_r = hT_d.rearrange("(k p) t -> p k t", p=P)
            wq_r = wqT_d.rearrange("(k p) r -> p k r", p=P)
            wk_r = wkT_d.rearrange("(k p) r -> p k r", p=P)
            nc.sync.dma_start(hT[:, 0:2, 0:512], hT_r[:, 0:2, 0:512])
            nc.scalar.dma_start(wqT[:, 0:4, :], wq_r[:, 0:4, :])
            nc.sync.dma_start(hT[:, 2:4, 0:512], hT_r[:, 2:4, 0:512])
            nc.scalar.dma_start(wqT[:, 4:8, :], wq_r[:, 4:8, :])
            nc.sync.dma_start(hT[:, 4:6, 0:512], hT_r[:, 4:6, 0:512])
            nc.scalar.dma_start(wkT[:, 0:4, :], wk_r[:, 0:4, :])
            nc.sync.dma_start(hT[:, 6:8, 0:512], hT_r[:, 6:8, 0:512])
            nc.scalar.dma_start(wkT[:, 4:8, :], wk_r[:, 4:8, :])
            nc.sync.dma_start(bq[:], bq_d.rearrange("(m p) one -> p (m one)", p=P))
            nc.sync.dma_start(bk[:], bk_d.rearrange("(m p) one -> p (m one)", p=P))
            nc.sync.dma_start(bv[:], bv_d[:])
            nc.vector.memset(vsb[:, :, :, HD : HD + 1], 1.0)

            def qkv_chunk(n):
                """QKV outputs for t-columns [n*512, (n+1)*512)."""
                nsl = slice(n * 512, (n + 1) * 512)
                if n + 1 < NCH:  # prefetch next chunk's hidden columns
                    psl = slice((n + 1) * 512, (n + 2) * 512)
                    for kk in range(0, KT, 2):
                        nc.sync.dma_start(
                            hT[:, kk : kk + 2, psl], hT_r[:, kk : kk + 2, psl]
                        )
                for w_sb, b_sb, o_sb in ((wqT, bq, qT), (wkT, bk, kT)):
                    for m in range(MT):
                        acc = ps.tile([P, 512], F32, tag="pv", bufs=4,
                                      name=f"qkacc_{n}_{m}")
                        for k in range(KT):
                            nc.tensor.matmul(
                                acc[:],
                                w_sb[:, k, m * P : (m + 1) * P],
                                hT[:, k, nsl],
                                start=(k == 0),
                                stop=(k == KT - 1),
                            )
                        # bias-add + bf16 cast on ACT: P0 is PE-bound and
                        # ACT has slack here; DVE keeps the f-mul stream.
                        nc.scalar.activation(
                            o_sb[:, m, nsl], acc[:], AF.Identity,
                            bias=b_sb[:, m : m + 1],
                        )
                if n == 0:
                    nc.sync.dma_start(
                        wvT[:], wvT_d.rearrange("(k p) r -> p k r", p=P)
                    )
                for s in range(4 * n, 4 * n + 4):
                    acc = ps.tile([P, R], F32, tag="pv", bufs=4, name=f"vacc_{s}")
                    for k in range(KT):
                        nc.tensor.matmul(
                            acc[:],
                            hT[:, k, s * P : (s + 1) * P],
                            wvT[:, k, :],
                            start=(k == 0),
                            stop=False,
                        )
                    nc.tensor.matmul(acc[:], ones_r[:], bv[:], start=False, stop=True)
                    nc.vector.tensor_copy(
                        vsb[:, s, :, 0:HD],
                        acc[:].rearrange("p (h d) -> p h d", h=HG),
                    )

            ft_tiles = {}

            def new_ft(tch):
                ft_tiles[tch] = sb.tile([P, ST, 512], BF16,
                                        tag=f"ft{tch % 2}", bufs=1, name=f"ft_t{tch}")

            def ft_load(tch, st):
                ftt = ft_tiles[tch]
                nc.sync.dma_start(
                    ftt[:, st, :],
                    fT_d[st * P : (st + 1) * P, tch * 512 : (tch + 1) * 512],
                )

            def live_step(pvp, st):
                """Unit u0 = (0,0): scores/exp/f-mul/PV chained directly."""
                sc = ps.tile([P, 1024], F32, tag="sc", bufs=2, name=f"sc00_{st}")
                e = sb.tile([P, 1024], BF16, tag="e", bufs=E_BUFS, name=f"e00_{st}")
                ssl = slice(st * P, (st + 1) * P)
                for a in range(2):
                    rows = slice(a * HD, (a + 1) * HD)
                    nc.tensor.matmul(
                        sc[:, a * 512 : (a + 1) * 512],
                        kT[rows, 0, ssl],
                        qT[rows, 0, 0:512],
                        start=True,
                        stop=True,
                    )
                nc.scalar.activation(e[:], sc[:], AF.Exp)
                for a in range(2):
                    half = slice(a * 512, (a + 1) * 512)
                    nc.vector.tensor_mul(e[:, half], e[:, half],
                                         ft_tiles[0][:, st, :])
                    nc.tensor.matmul(
                        pvp[a][:],
                        vsb[:, st, a, :],
                        e[:, half],
                        start=(st == 0),
                        stop=(st == ST - 1),
                    )

            def front_st(u, st):
                """Scores/exp/f-mul for unit u, e stashed in SBUF."""
                tch, j = divmod(u, 2)
                S = stash[u % 2]
                tsl = slice(tch * 512, (tch + 1) * 512)
                ssl = slice(st * P, (st + 1) * P)
                sc = ps.tile([P, 1024], F32, tag="sc", bufs=2, name=f"sc{u}_{st}")
                for a in range(2):
                    rows = slice(a * HD, (a + 1) * HD)
                    nc.tensor.matmul(
                        sc[:, a * 512 : (a + 1) * 512],
                        kT[rows, j, ssl],
                        qT[rows, j, tsl],
                        start=True,
                        stop=True,
                    )
                nc.scalar.activation(S[:, st, :], sc[:], AF.Exp)
                for a in range(2):
                    nc.vector.tensor_mul(
                        S[:, st, a * 512 : (a + 1) * 512],
                        S[:, st, a * 512 : (a + 1) * 512],
                        ft_tiles[tch][:, st, :],
                    )

            def back_st(u, pvp, st):
                """PV accumulation step for unit u from its stash."""
                S = stash[u % 2]
                tch, j = divmod(u, 2)
                for a in range(2):
                    nc.tensor.matmul(
                        pvp[a][:],
                        vsb[:, st, 2 * j + a, :],
                        S[:, st, a * 512 : (a + 1) * 512],
                        start=(st == 0),
                        stop=(st == ST - 1),
                    )

            pending_norms = []

            def norm_fast(u, pvp):
                """Free the PSUM accumulators: praw copies and the rowsum
                reciprocal, all on DVE (ACT stays exp-pure); scale deferred."""
                tch, j = divmod(u, 2)
                for a in range(2):
                    h = 2 * j + a
                    recip = sb.tile([1, 512], BF16, tag="recip", bufs=4,
                                    name=f"recip_{tch}_{h}")
                    praw = sb.tile([HD, 512], BF16, tag="praw", bufs=4,
                                   name=f"praw_{tch}_{h}")
                    nc.vector.tensor_copy(praw[:], pvp[a][0:HD, :])
                    nc.vector.reciprocal(recip[:], pvp[a][HD : HD + 1, :])
                    pending_norms.append((tch, h, recip, praw))

            def norm_defer():
                """Broadcast 1/rowsum across partitions (Pool) and scale."""
                while pending_norms:
                    tch, h, recip, praw = pending_norms.pop(0)
                    bcs = sb.tile([HD, 512], BF16, tag="bcs", bufs=4,
                                  name=f"bcs_{tch}_{h}")
                    nc.gpsimd.partition_broadcast(bcs[:], recip[:])
                    nc.vector.tensor_mul(
                        po[(h % 2) * HD : (h % 2) * HD + HD, h // 2,
                           tch * 512 : (tch + 1) * 512],
                        praw[:],
                        bcs[:],
                    )

            def outproj_unit(u, copy_eng):
                tt, n = divmod(u, 2)
                fin = ps.tile([P, 512], F32, tag="pv", bufs=4, name=f"fin_{tt}_{n}")
                osb = sb.tile([P, 512], BF16, tag="osb", bufs=3, name=f"osb_{tt}_{n}")
                for j in range(MT):
                    nc.tensor.matmul(
                        fin[:],
                        po[:, j, tt * P : (tt + 1) * P],
                        woT[:, j, n * 512 : (n + 1) * 512],
                        start=(j == 0),
                        stop=(j == MT - 1),
                    )
                if copy_eng == "act":
                    nc.scalar.copy(osb[:], fin[:])
                else:
                    nc.vector.tensor_copy(osb[:], fin[:])
                nc.sync.dma_start(
                    out_d[tt * P : (tt + 1) * P, n * 512 : (n + 1) * 512], osb[:]
                )

            def new_pv_pair(u):
                return [ps.tile([HD + 1, 512], F32, tag="pv", bufs=4,
                                name=f"pv_{u}_{a}") for a in range(2)]

            # ---- emission ----
            # P0: QKV chunks with u0 live plus fronts u1 and u2 (as the kT
            # s-tiles they need appear); ~48 of 128 exps land here, matching
            # ACT:PE ratios per phase.
            new_ft(0)
            new_ft(1)
            pv00 = new_pv_pair(0)
            for n in range(NCH):
                qkv_chunk(n)
                for st in range(4 * n, 4 * n + 4):
                    ft_load(0, st)
                    ft_load(1, st)
                live_step_range = range(4 * n, 4 * n + 4)
                for st in live_step_range:
                    live_step(pv00, st)
                for st in range(4 * n, 4 * n + 4):
                    front_st(1, st)
                if n > 0:
                    for st in range(4 * (n - 1), 4 * n):
                        front_st(2, st)
            for st in range(12, ST):
                front_st(2, st)
            norm_fast(0, pv00)
            nc.sync.dma_start(woT[:], woT_d.rearrange("(m p) d -> p m d", p=P))

            # segments k=1..7: back(k) + front(k+2) + out-proj bursts.
            # burst budget per segment: t-chunk X (units 8X..8X+7) is ready
            # in segment 2X+2 (normed+deferred); 6 units then, 2 in the next.
            burst_sched = {2: (0, 0, 6), 3: (0, 6, 8), 4: (1, 0, 6),
                           5: (1, 6, 8), 6: (2, 0, 6), 7: (2, 6, 8)}
            for k in range(1, 8):
                fr = k + 2 if k + 2 <= 7 else None
                if fr is not None:
                    ftch = fr // 2
                    if ftch not in ft_tiles:
                        new_ft(ftch)
                        for st in range(ST):
                            ft_load(ftch, st)
                norm_defer()
                pvk = new_pv_pair(k)
                burst = burst_sched.get(k)
                burst_units = []
                if burst is not None:
                    btch, lo, hi = burst
                    burst_units = [8 * btch + i for i in range(lo, hi)]
                copy_eng = "dve" if k <= 5 else "act"
                for st in range(ST):
                    back_st(k, pvk, st)
                    if fr is not None:
                        front_st(fr, st)
                    if burst_units and st >= 2 and st % 2 == 0:
                        bu = burst_units.pop(0)
                        outproj_unit(bu, copy_eng)
                for bu in burst_units:
                    outproj_unit(bu, copy_eng)
                norm_fast(k, pvk)

            # tail: t-chunk 3 out-projections.
            norm_defer()
            for u in range(24, 32):
                outproj_unit(u, "act" if u % 2 else "dve")

    return nc


_NC = None
_LAST_RESULT = None


def _get_nc():
    global _NC
    if _NC is None:
        _NC = build_bass()
        if not _NC.is_finalized():
            _NC.finalize()
    return _NC


def kernel(hidden_states, focused_attention, Wq, bq, Wk, bk, Wv, bv, Wo, bo):
    bf = ml_dtypes.bfloat16
    hT = [np.ascontiguousarray(hidden_states[b].T).astype(bf) for b in range(B)]
    fT = [np.ascontiguousarray(focused_attention[b].T).astype(bf) for b in range(B)]

    in_maps = []
    for c in range(N_CORES):
        b, g = divmod(c, 4)
        rows = slice(g * R, (g + 1) * R)
        in_maps.append({
            "hT": hT[b],
            "fT": fT[b],
            "wqT": np.ascontiguousarray((Wq[rows] * SCALING).T).astype(bf),
            "wkT": np.ascontiguousarray(Wk[rows].T).astype(bf),
            "wvT": np.ascontiguousarray(Wv[rows].T).astype(bf),
            "woT": np.ascontiguousarray(Wo[:, rows].T).astype(bf),
            "bq": np.ascontiguousarray((bq[rows] * SCALING)[:, None]).astype(np.float32),
            "bk": np.ascontiguousarray(bk[rows][:, None]).astype(np.float32),
            "bv": np.ascontiguousarray(bv[rows][None, :]).astype(bf),
        })

    res = run_bass_kernel_spmd(_get_nc(), in_maps, list(range(N_CORES)))
    global _LAST_RESULT
    _LAST_RESULT = res
    out = np.zeros((B, T, D), dtype=np.float32)
    for c in range(N_CORES):
        out[c // 4] += np.asarray(res.results[c]["out_partial"], dtype=np.float32)
    out += np.asarray(bo, dtype=np.float32)[None, None, :]
    return out


# revision 11
# speedup vs baseline: 1.1447x; 1.0292x over previous
"""BartAttention (focused-attention variant) Trainium2 Bass kernel.

Problem (hardcoded): B=2, T=2048, D=1024, H=16 heads, hd=64.
  q = (h @ Wq.T + bq) * hd**-0.5 ; k = h @ Wk.T + bk ; v = h @ Wv.T + bv
  scores = q @ k.T per head ; e = f * exp(scores) ; attn = e / rowsum(e)
  out = (attn @ v) @ Wo.T + bo

Sharding over 8 cores: batch (2) x head-group (4 groups of 4 heads).
Each core computes its heads' QKV, attention, and a partial out-projection
(contraction over its 256 d-columns of Wo); host sums the 4 bf16 partials
per batch in f32 and adds bo.

On-device layout (per core):
  hT   [1024, 2048] bf16   hidden.T               (c on partitions)
  qT,kT [256, 2048] bf16   q.T / k.T              (head*hd on partitions)
  v    [2048, 4, 65] bf16  v per head + ones col  (s on partitions)
  scores computed transposed: sT[s,t] = k @ q.T so that e=f.T*exp(sT) has
  s on partitions, which is the contraction dim of the PV matmul.
  PV: outT_aug[65, t] = [v | 1].T @ e  -> row 64 = rowsum(e) per t (exact fp32).
  out-proj: final[t, m] = outT.T @ Wo_slice.T, scaled per head by 1/rowsum
  (1/rowsum broadcast across partitions by GPSIMD, multiply on DVE at 2x).

The 8 head-pair units (tch, j) split into a "front" (scores -> exp ->
f-mul, needs only the 2 `sc` PSUM tiles; all units but u0 stash the result
in one of two SBUF e-stashes, u0 chains its PV live) and a "back" (16 PV
accumulation steps from the stash, needs only 2 `pv` PSUM banks).

Emission is driven by a virtual-clock list scheduler: per-engine virtual
times (PE/ACT/DVE/Pool) advance as instructions are emitted, a front step
is emitted only when ACT would otherwise run dry AND an `sc` slot will be
free when its scores reach the PE queue head (engine queues are in-order,
so an emitted-but-blocked instruction stalls everything behind it), and
otherwise ACT-independent PE filler is emitted: QKV chains during the
first quarter, then PV back-steps and out-projection bursts.  This keeps
the PE queue free of head-of-line blocking, which is what limited
phase-ordered emission.

Engine placement: PE matmuls ~165us (binding); ACT = exps + q/k bias adds
+ some out-proj staging; DVE = f-muls, v copy, reciprocal, praw copies,
po scale-muls, rest of out-proj staging; Pool = 1/rowsum broadcasts.
"""

import numpy as np
import ml_dtypes

import concourse.bass as bass
import concourse.bacc as bacc
import concourse.mybir as mybir
from concourse.tile import TileContext
from concourse.bass_utils import run_bass_kernel_spmd

BF16 = mybir.dt.bfloat16
F32 = mybir.dt.float32
AF = mybir.ActivationFunctionType

B, T, D = 2, 2048, 1024
H, HD = 16, 64
HG = 4               # heads per core
R = HG * HD          # 256 d-rows per core
SCALING = HD ** -0.5
N_CORES = 8

P = 128
KT = D // P          # 8 k-tiles for QKV contraction
MT = R // P          # 2 m-tiles of qT/kT rows
NCH = T // 512       # 4 chunks of 512 along t
ST = T // P          # 16 s-tiles

E_BUFS = 5

# virtual-clock costs (ns), from the TRN2 cost model
MM512 = 512 * (1e9 / 2.4e9)          # one 512-col bf16 matmul
MM256 = 256 * (1e9 / 2.4e9)
EXP = (1024 + 222) * (1e9 / 1.2e9)   # [128,1024] exp, PSUM->SBUF
BIAS = (512 + 222) * (1e9 / 1.2e9)   # bias-add copy on ACT
FMUL = (256 + 58) * (1e9 / 0.96e9)   # [128,512] bf16 mul at DVE 2x
VCOPY = (256 + 120) * (1e9 / 0.96e9)
PRAW = (512 + 120) * (1e9 / 0.96e9)
RECIP = (512 + 120) * (1e9 / 0.96e9)
POMUL = (256 + 58) * (1e9 / 0.96e9)
OSB_DVE = (512 + 120) * (1e9 / 0.96e9)
OSB_ACT = (512 + 222) * (1e9 / 1.2e9)
BCAST = 512 * (1e9 / 1.2e9) + 95
ACT_OH = 60.0                        # dispatch/seq overheads per ACT instr
LOOKAHEAD = 1500.0                   # emit front when ACT lead < this


def build_bass():
    nc = bacc.Bacc()

    hT_d = nc.declare_dram_parameter("hT", [D, T], BF16, isOutput=False)
    fT_d = nc.declare_dram_parameter("fT", [T, T], BF16, isOutput=False)
    wqT_d = nc.declare_dram_parameter("wqT", [D, R], BF16, isOutput=False)
    wkT_d = nc.declare_dram_parameter("wkT", [D, R], BF16, isOutput=False)
    wvT_d = nc.declare_dram_parameter("wvT", [D, R], BF16, isOutput=False)
    woT_d = nc.declare_dram_parameter("woT", [R, D], BF16, isOutput=False)
    bq_d = nc.declare_dram_parameter("bq", [R, 1], F32, isOutput=False)
    bk_d = nc.declare_dram_parameter("bk", [R, 1], F32, isOutput=False)
    bv_d = nc.declare_dram_parameter("bv", [1, R], BF16, isOutput=False)
    out_d = nc.declare_dram_parameter("out_partial", [T, D], BF16, isOutput=True)

    with TileContext(nc) as tc:
        with (
            nc.allow_low_precision(reason="bf16 pipeline is intentional"),
            tc.tile_pool(name="sb", bufs=1) as sb,
            tc.tile_pool(name="ps", bufs=1, space="PSUM") as ps,
        ):
            # ---- persistent SBUF tensors ----
            hT = sb.tile([P, KT, T], BF16)
            wqT = sb.tile([P, KT, R], BF16)
            wkT = sb.tile([P, KT, R], BF16)
            wvT = sb.tile([P, KT, R], BF16)
            woT = sb.tile([P, MT, D], BF16)
            bq = sb.tile([P, MT], F32)
            bk = sb.tile([P, MT], F32)
            bv = sb.tile([1, R], BF16)
            ones_r = sb.tile([1, P], BF16)     # K=1 lhsT for v-bias matmul
            qT = sb.tile([P, MT, T], BF16)
            kT = sb.tile([P, MT, T], BF16)
            vsb = sb.tile([P, ST, HG, HD + 1], BF16)
            po = sb.tile([P, MT, T], BF16)     # scaled outT, out-proj lhsT
            stash = [sb.tile([P, ST, 1024], BF16, name=f"stash{i}")
                     for i in range(2)]

            # warmup: a 1-column matmul as soon as ones_r is set starts the
            # PE p-state clock, so real matmuls (arriving ~4us later, past
            # the 3us ramp) run at full clock from the first chunk.
            nc.vector.memset(ones_r[:], 1.0)
            warm = ps.tile([1, 1], F32, tag="pv", bufs=4, name="warm")
            nc.tensor.matmul(warm[:], ones_r[:, 0:1], ones_r[:, 0:1],
                             start=True, stop=True)

            # startup DMAs: first hT chunk on the SP queue; q/k weights in
            # k-halves on the ACT HWDGE queue, interleaved so the first QKV
            # matmuls get their operands as early as possible.
            hT_r = hT_d.rearrange("(k p) t -> p k t", p=P)
            wq_r = wqT_d.rearrange("(k p) r -> p k r", p=P)
            wk_r = wkT_d.rearrange("(k p) r -> p k r", p=P)
            nc.sync.dma_start(hT[:, 0:2, 0:512], hT_r[:, 0:2, 0:512])
            nc.scalar.dma_start(wqT[:, 0:4, :], wq_r[:, 0:4, :])
            nc.sync.dma_start(hT[:, 2:4, 0:512], hT_r[:, 2:4, 0:512])
            nc.scalar.dma_start(wqT[:, 4:8, :], wq_r[:, 4:8, :])
            nc.sync.dma_start(hT[:, 4:6, 0:512], hT_r[:, 4:6, 0:512])
            nc.scalar.dma_start(wkT[:, 0:4, :], wk_r[:, 0:4, :])
            nc.sync.dma_start(hT[:, 6:8, 0:512], hT_r[:, 6:8, 0:512])
            nc.scalar.dma_start(wkT[:, 4:8, :], wk_r[:, 4:8, :])
            nc.sync.dma_start(bq[:], bq_d.rearrange("(m p) one -> p (m one)", p=P))
            nc.sync.dma_start(bk[:], bk_d.rearrange("(m p) one -> p (m one)", p=P))
            nc.sync.dma_start(bv[:], bv_d[:])
            nc.vector.memset(vsb[:, :, :, HD : HD + 1], 1.0)

            ft_tiles = {}
            emitted_ft = set()

            def new_ft(tch):
                ft_tiles[tch] = sb.tile([P, ST, 512], BF16,
                                        tag=f"ft{tch % 2}", bufs=1, name=f"ft_t{tch}")

            def ft_need(tch, st):
                """Ensure f tiles for (tch, st..st+2) are loading."""
                if tch not in ft_tiles:
                    new_ft(tch)
                for s in range(st, min(st + 3, ST)):
                    if (tch, s) not in emitted_ft:
                        emitted_ft.add((tch, s))
                        nc.sync.dma_start(
                            ft_tiles[tch][:, s, :],
                            fT_d[s * P : (s + 1) * P,
                                 tch * 512 : (tch + 1) * 512],
                        )

            # ================= virtual-clock scheduler =================
            clk = {"pe": 0.0, "act": 0.0, "dve": 0.0, "pool": 0.0}

            exp_done = {}     # (u, st) -> ACT virtual completion of exp
            fmul_done = {}    # (u, st) -> DVE virtual completion of f-mul
            qk_ready = {}     # (w, n, m) -> ACT completion of bias-add
            v_ready = {}      # st -> DVE completion of v copy-out
            po_ready = {}     # u -> DVE completion of both po scale-muls
            pv_done = {}      # (u, st) -> PE completion of PV step
            sc_queue = []     # ACT completion times of in-flight exps
            e_tiles = {}      # st -> e tile for live u0
            pv_pairs = {}
            norm_hold = []    # (u, h, recip, praw, recip_done)

            def new_pv_pair(u):
                pv_pairs[u] = [ps.tile([HD + 1, 512], F32, tag="pv", bufs=4,
                                       name=f"pv_{u}_{a}") for a in range(2)]

            def emit_chain(w, n, m):
                w_sb, b_sb, o_sb = ((wqT, bq, qT) if w == "q" else (wkT, bk, kT))
                nsl = slice(n * 512, (n + 1) * 512)
                acc = ps.tile([P, 512], F32, tag="pv", bufs=4,
                              name=f"{w}acc_{n}_{m}")
                for k in range(KT):
                    nc.tensor.matmul(
                        acc[:], w_sb[:, k, m * P : (m + 1) * P], hT[:, k, nsl],
                        start=(k == 0), stop=(k == KT - 1),
                    )
                clk["pe"] += KT * MM512
                nc.scalar.activation(o_sb[:, m, nsl], acc[:], AF.Identity,
                                     bias=b_sb[:, m : m + 1])
                clk["act"] = max(clk["act"], clk["pe"]) + BIAS + ACT_OH
                qk_ready[(w, n, m)] = clk["act"]

            def emit_vchain(st):
                acc = ps.tile([P, R], F32, tag="pv", bufs=4, name=f"vacc_{st}")
                for k in range(KT):
                    nc.tensor.matmul(
                        acc[:], hT[:, k, st * P : (st + 1) * P], wvT[:, k, :],
                        start=(k == 0), stop=False,
                    )
                nc.tensor.matmul(acc[:], ones_r[:], bv[:], start=False, stop=True)
                clk["pe"] += (KT + 1) * MM256
                nc.vector.tensor_copy(
                    vsb[:, st, :, 0:HD],
                    acc[:].rearrange("p (h d) -> p h d", h=HG),
                )
                clk["dve"] = max(clk["dve"], clk["pe"]) + VCOPY
                v_ready[st] = clk["dve"]

            def emit_front(u, st):
                tch, j = divmod(u, 2)
                ft_need(tch, st)
                tsl = slice(tch * 512, (tch + 1) * 512)
                ssl = slice(st * P, (st + 1) * P)
                sc = ps.tile([P, 1024], F32, tag="sc", bufs=2, name=f"sc{u}_{st}")
                for a in range(2):
                    rows = slice(a * HD, (a + 1) * HD)
                    nc.tensor.matmul(
                        sc[:, a * 512 : (a + 1) * 512],
                        kT[rows, j, ssl], qT[rows, j, tsl],
                        start=True, stop=True,
                    )
                clk["pe"] = max(clk["pe"], qk_ready[("q", tch, j)],
                                qk_ready[("k", st // 4, j)]) + 2 * MM512
                if u == 0:
                    dst = sb.tile([P, 1024], BF16, tag="e", bufs=E_BUFS,
                                  name=f"e00_{st}")
                    e_tiles[st] = dst
                else:
                    dst = stash[u % 2][:, st, :]
                nc.scalar.activation(dst, sc[:], AF.Exp)
                war = pv_done.get((u - 2, st), 0.0) if u >= 3 else 0.0
                clk["act"] = max(clk["act"], clk["pe"], war) + EXP + ACT_OH
                exp_done[(u, st)] = clk["act"]
                sc_queue.append(clk["act"])
                for a in range(2):
                    half = slice(a * 512, (a + 1) * 512)
                    nc.vector.tensor_mul(dst[:, half], dst[:, half],
                                         ft_tiles[tch][:, st, :])
                clk["dve"] = max(clk["dve"], clk["act"]) + 2 * FMUL
                fmul_done[(u, st)] = clk["dve"]

            def emit_pv(u, st):
                """PV step: u0 from its e tile, others from the stash."""
                tch, j = divmod(u, 2)
                if u == 0:
                    src = e_tiles.pop(st)
                else:
                    src = stash[u % 2][:, st, :]
                for a in range(2):
                    nc.tensor.matmul(
                        pv_pairs[u][a][:], vsb[:, st, 2 * j + a, :],
                        src[:, a * 512 : (a + 1) * 512],
                        start=(st == 0), stop=(st == ST - 1),
                    )
                clk["pe"] = max(clk["pe"], fmul_done[(u, st)], v_ready[st])
                clk["pe"] += 2 * MM512
                pv_done[(u, st)] = clk["pe"]
                if st == ST - 1:
                    emit_norm(u)

            def emit_norm(u):
                tch, j = divmod(u, 2)
                pvp = pv_pairs[u]
                for a in range(2):
                    h = 2 * j + a
                    recip = sb.tile([1, 512], BF16, tag="recip", bufs=4,
                                    name=f"recip_{tch}_{h}")
                    praw = sb.tile([HD, 512], BF16, tag="praw", bufs=4,
                                   name=f"praw_{tch}_{h}")
                    nc.vector.tensor_copy(praw[:], pvp[a][0:HD, :])
                    nc.vector.reciprocal(recip[:], pvp[a][HD : HD + 1, :])
                    clk["dve"] = max(clk["dve"], pv_done[(u, ST - 1)])
                    clk["dve"] += PRAW + RECIP
                    norm_hold.append((u, h, recip, praw, clk["dve"]))

            def flush_norms(force=False):
                while norm_hold:
                    u, h, recip, praw, rdone = norm_hold[0]
                    pool_t = max(clk["pool"], rdone) + BCAST
                    if not force and pool_t > clk["dve"] + 400:
                        break
                    norm_hold.pop(0)
                    clk["pool"] = pool_t
                    tch = u // 2
                    bcs = sb.tile([HD, 512], BF16, tag="bcs", bufs=4,
                                  name=f"bcs_{tch}_{h}")
                    nc.gpsimd.partition_broadcast(bcs[:], recip[:])
                    nc.vector.tensor_mul(
                        po[(h % 2) * HD : (h % 2) * HD + HD, h // 2,
                           tch * 512 : (tch + 1) * 512],
                        praw[:], bcs[:],
                    )
                    clk["dve"] = max(clk["dve"], clk["pool"]) + POMUL
                    po_ready[u] = max(po_ready.get(u, 0.0), clk["dve"])

            def emit_burst(i):
                tt, n = divmod(i, 2)
                tch = tt // 4
                fin = ps.tile([P, 512], F32, tag="pv", bufs=4, name=f"fin_{tt}_{n}")
                osb = sb.tile([P, 512], BF16, tag="osb", bufs=3, name=f"osb_{tt}_{n}")
                gate = max(po_ready[2 * tch], po_ready[2 * tch + 1])
                for j in range(MT):
                    nc.tensor.matmul(
                        fin[:], po[:, j, tt * P : (tt + 1) * P],
                        woT[:, j, n * 512 : (n + 1) * 512],
                        start=(j == 0), stop=(j == MT - 1),
                    )
                clk["pe"] = max(clk["pe"], gate) + 2 * MM512
                if clk["act"] <= clk["dve"]:
                    nc.scalar.copy(osb[:], fin[:])
                    clk["act"] = max(clk["act"], clk["pe"]) + OSB_ACT + ACT_OH
                else:
                    nc.vector.tensor_copy(osb[:], fin[:])
                    clk["dve"] = max(clk["dve"], clk["pe"]) + OSB_DVE
                nc.sync.dma_start(
                    out_d[tt * P : (tt + 1) * P, n * 512 : (n + 1) * 512], osb[:]
                )

            # ---------- work lists ----------
            chain_list = []
            for n in range(NCH):
                for m in range(MT):
                    chain_list.append(("q", n, m))
                    chain_list.append(("k", n, m))
                for st in range(4 * n, 4 * n + 4):
                    chain_list.append(("v", st))

            front_list = []
            for n in range(NCH):
                for st in range(4 * n, 4 * n + 4):
                    front_list.append((0, st))
                    front_list.append((1, st))
                if n >= 1:
                    for st in range(4 * (n - 1), 4 * n):
                        front_list.append((2, st))
            for st in range(12, ST):
                front_list.append((2, st))
            for u in range(3, 8):
                for st in range(ST):
                    front_list.append((u, st))

            # PV steps: u0 in P0 (live), then u1..u7 sequentially.
            pv_list = [(0, st) for st in range(ST)]
            pv_list += [(u, st) for u in range(1, 8) for st in range(ST)]
            burst_list = list(range(32))

            ci = fi = bi = gi = 0
            emitted_wo = False

            def front_ok(strict):
                if fi >= len(front_list):
                    return False
                u, st = front_list[fi]
                tch, j = divmod(u, 2)
                if ("q", tch, j) not in qk_ready:
                    return False
                if ("k", st // 4, j) not in qk_ready:
                    return False
                if u >= 3 and (u - 2, st) not in pv_done:
                    return False  # stash WAR: hard correctness dependency
                if not strict:
                    return True
                pend = sum(1 for t in sc_queue[-2:] if t > clk["pe"])
                return pend < 2

            def pv_ok(strict):
                if bi >= len(pv_list):
                    return False
                u, st = pv_list[bi]
                if (u, st) not in fmul_done or st not in v_ready:
                    return False
                if u == 1 and ci < len(chain_list):
                    return False  # PSUM: chains still rotating pv slots
                if u >= 1 and (u - 1 if u > 1 else 0, ST - 1) not in pv_done:
                    return False  # one stash-back pair at a time
                if not strict:
                    return True
                return fmul_done[(u, st)] <= clk["pe"] + 300

            def burst_ok(strict):
                if gi >= len(burst_list) or not emitted_wo:
                    return False
                tch = burst_list[gi] // 8
                if 2 * tch not in po_ready or 2 * tch + 1 not in po_ready:
                    return False
                if not strict:
                    return True
                return max(po_ready[2 * tch], po_ready[2 * tch + 1]) \
                    <= clk["pe"] + 300

            new_pv_pair(0)

            while (ci < len(chain_list) or fi < len(front_list)
                   or bi < len(pv_list) or gi < len(burst_list) or norm_hold):
                flush_norms()

                if clk["act"] - clk["pe"] < LOOKAHEAD and front_ok(True):
                    u, st = front_list[fi]
                    fi += 1
                    emit_front(u, st)
                    continue
                if pv_ok(True):
                    u, st = pv_list[bi]
                    bi += 1
                    if u not in pv_pairs:
                        new_pv_pair(u)
                    emit_pv(u, st)
                    continue
                if ci < len(chain_list):
                    item = chain_list[ci]
                    ci += 1
                    if item[0] == "v":
                        if item[1] == 0:
                            nc.sync.dma_start(
                                wvT[:],
                                wvT_d.rearrange("(k p) r -> p k r", p=P))
                        emit_vchain(item[1])
                    else:
                        w, n, m = item
                        if w == "q" and m == 0 and n + 1 < NCH:
                            psl = slice((n + 1) * 512, (n + 2) * 512)
                            for kk in range(0, KT, 2):
                                nc.sync.dma_start(
                                    hT[:, kk : kk + 2, psl],
                                    hT_r[:, kk : kk + 2, psl])
                        emit_chain(w, n, m)
                    if ci == len(chain_list):
                        nc.sync.dma_start(
                            woT[:], woT_d.rearrange("(m p) d -> p m d", p=P))
                        emitted_wo = True
                    continue
                if burst_ok(True):
                    emit_burst(burst_list[gi])
                    gi += 1
                    continue

                # nothing cleanly ready: emit the least-stalling item
                if pv_ok(False):
                    u, st = pv_list[bi]
                    bi += 1
                    if u not in pv_pairs:
                        new_pv_pair(u)
                    emit_pv(u, st)
                elif burst_ok(False):
                    emit_burst(burst_list[gi])
                    gi += 1
                elif front_ok(False):
                    u, st = front_list[fi]
                    fi += 1
                    emit_front(u, st)
                elif norm_hold:
                    flush_norms(force=True)
                else:
                    raise RuntimeError(
                        f"scheduler wedged: ci={ci} fi={fi} bi={bi} gi={gi}")

    return nc


_NC = None
_LAST_RESULT = None


def _get_nc():
    global _NC
    if _NC is None:
        _NC = build_bass()
        if not _NC.is_finalized():
            _NC.finalize()
    return _NC


def kernel(hidden_states, focused_attention, Wq, bq, Wk, bk, Wv, bv, Wo, bo):
    bf = ml_dtypes.bfloat16
    hT = [np.ascontiguousarray(hidden_states[b].T).astype(bf) for b in range(B)]
    fT = [np.ascontiguousarray(focused_attention[b].T).astype(bf) for b in range(B)]

    in_maps = []
    for c in range(N_CORES):
        b, g = divmod(c, 4)
        rows = slice(g * R, (g + 1) * R)
        in_maps.append({
            "hT": hT[b],
            "fT": fT[b],
            "wqT": np.ascontiguousarray((Wq[rows] * SCALING).T).astype(bf),
            "wkT": np.ascontiguousarray(Wk[rows].T).astype(bf),
            "wvT": np.ascontiguousarray(Wv[rows].T).astype(bf),
            "woT": np.ascontiguousarray(Wo[:, rows].T).astype(bf),
            "bq": np.ascontiguousarray((bq[rows] * SCALING)[:, None]).astype(np.float32),
            "bk": np.ascontiguousarray(bk[rows][:, None]).astype(np.float32),
            "bv": np.ascontiguousarray(bv[rows][None, :]).astype(bf),
        })

    res = run_bass_kernel_spmd(_get_nc(), in_maps, list(range(N_CORES)))
    global _LAST_RESULT
    _LAST_RESULT = res
    out = np.zeros((B, T, D), dtype=np.float32)
    for c in range(N_CORES):
        out[c // 4] += np.asarray(res.results[c]["out_partial"], dtype=np.float32)
    out += np.asarray(bo, dtype=np.float32)[None, None, :]
    return out


# revision 13
# speedup vs baseline: 1.1528x; 1.0070x over previous
"""BartAttention (focused-attention variant) Trainium2 Bass kernel.

Problem (hardcoded): B=2, T=2048, D=1024, H=16 heads, hd=64.
  q = (h @ Wq.T + bq) * hd**-0.5 ; k = h @ Wk.T + bk ; v = h @ Wv.T + bv
  scores = q @ k.T per head ; e = f * exp(scores) ; attn = e / rowsum(e)
  out = (attn @ v) @ Wo.T + bo

Sharding over 8 cores: batch (2) x head-group (4 groups of 4 heads).
Each core computes its heads' QKV, attention, and a partial out-projection
(contraction over its 256 d-columns of Wo); host sums the 4 bf16 partials
per batch in f32 and adds bo.

On-device layout (per core):
  hT   [1024, 2048] bf16   hidden.T               (c on partitions)
  qT,kT [256, 2048] bf16   q.T / k.T              (head*hd on partitions)
  v    [2048, 4, 65] bf16  v per head + ones col  (s on partitions)
  scores computed transposed: sT[s,t] = k @ q.T so that e=f.T*exp(sT) has
  s on partitions, which is the contraction dim of the PV matmul.
  PV: outT_aug[65, t] = [v | 1].T @ e  -> row 64 = rowsum(e) per t (exact fp32).
  out-proj: final[t, m] = outT.T @ Wo_slice.T, scaled per head by 1/rowsum
  (1/rowsum broadcast across partitions by GPSIMD, multiply on DVE at 2x).

The 8 head-pair units (tch, j) split into a "front" (scores -> exp ->
f-mul, needs only the 2 `sc` PSUM tiles; all units but u0 stash the result
in one of two SBUF e-stashes, u0 chains its PV live) and a "back" (16 PV
accumulation steps from the stash, needs only 2 `pv` PSUM banks).

Emission is driven by a virtual-clock list scheduler: per-engine virtual
times (PE/ACT/DVE/Pool) advance as instructions are emitted, a front step
is emitted only when ACT would otherwise run dry AND an `sc` slot will be
free when its scores reach the PE queue head (engine queues are in-order,
so an emitted-but-blocked instruction stalls everything behind it), and
otherwise ACT-independent PE filler is emitted: QKV chains during the
first quarter, then PV back-steps and out-projection bursts.  This keeps
the PE queue free of head-of-line blocking, which is what limited
phase-ordered emission.

Engine placement: PE matmuls ~165us (binding); ACT = exps + q/k bias adds
+ some out-proj staging; DVE = f-muls, v copy, reciprocal, praw copies,
po scale-muls, rest of out-proj staging; Pool = 1/rowsum broadcasts.
"""

import numpy as np
import ml_dtypes

import concourse.bass as bass
import concourse.bacc as bacc
import concourse.mybir as mybir
from concourse.tile import TileContext
from concourse.bass_utils import run_bass_kernel_spmd

BF16 = mybir.dt.bfloat16
F32 = mybir.dt.float32
AF = mybir.ActivationFunctionType

B, T, D = 2, 2048, 1024
H, HD = 16, 64
HG = 4               # heads per core
R = HG * HD          # 256 d-rows per core
SCALING = HD ** -0.5
N_CORES = 8

P = 128
KT = D // P          # 8 k-tiles for QKV contraction
MT = R // P          # 2 m-tiles of qT/kT rows
NCH = T // 512       # 4 chunks of 512 along t
ST = T // P          # 16 s-tiles

E_BUFS = 5

# virtual-clock costs (ns), from the TRN2 cost model
MM512 = 512 * (1e9 / 2.4e9)          # one 512-col bf16 matmul
MM256 = 256 * (1e9 / 2.4e9)
EXP = (1024 + 222) * (1e9 / 1.2e9)   # [128,1024] exp, PSUM->SBUF
BIAS = (512 + 222) * (1e9 / 1.2e9)   # bias-add copy on ACT
FMUL = (256 + 58) * (1e9 / 0.96e9)   # [128,512] bf16 mul at DVE 2x
VCOPY = (256 + 120) * (1e9 / 0.96e9)
PRAW = (512 + 120) * (1e9 / 0.96e9)
RECIP = (512 + 120) * (1e9 / 0.96e9)
POMUL = (256 + 58) * (1e9 / 0.96e9)
OSB_DVE = (512 + 120) * (1e9 / 0.96e9)
OSB_ACT = (512 + 222) * (1e9 / 1.2e9)
BCAST = 512 * (1e9 / 1.2e9) + 95
ACT_OH = 60.0                        # dispatch/seq overheads per ACT instr
LOOKAHEAD = 1500.0                   # emit front when ACT lead < this


def build_bass():
    nc = bacc.Bacc()

    hT_d = nc.declare_dram_parameter("hT", [D, T], BF16, isOutput=False)
    fT_d = nc.declare_dram_parameter("fT", [T, T], BF16, isOutput=False)
    wqT_d = nc.declare_dram_parameter("wqT", [D, R], BF16, isOutput=False)
    wkT_d = nc.declare_dram_parameter("wkT", [D, R], BF16, isOutput=False)
    wvT_d = nc.declare_dram_parameter("wvT", [D, R], BF16, isOutput=False)
    woT_d = nc.declare_dram_parameter("woT", [R, D], BF16, isOutput=False)
    bq_d = nc.declare_dram_parameter("bq", [R, 1], F32, isOutput=False)
    bk_d = nc.declare_dram_parameter("bk", [R, 1], F32, isOutput=False)
    bv_d = nc.declare_dram_parameter("bv", [1, R], BF16, isOutput=False)
    out_d = nc.declare_dram_parameter("out_partial", [T, D], BF16, isOutput=True)

    with TileContext(nc) as tc:
        with (
            nc.allow_low_precision(reason="bf16 pipeline is intentional"),
            tc.tile_pool(name="sb", bufs=1) as sb,
            tc.tile_pool(name="ps", bufs=1, space="PSUM") as ps,
        ):
            # ---- persistent SBUF tensors ----
            hT = sb.tile([P, KT, T], BF16)
            wqT = sb.tile([P, KT, R], BF16)
            wkT = sb.tile([P, KT, R], BF16)
            wvT = sb.tile([P, KT, R], BF16)
            woT = sb.tile([P, MT, D], BF16)
            bq = sb.tile([P, MT], F32)
            bk = sb.tile([P, MT], F32)
            bv = sb.tile([1, R], BF16)
            ones_r = sb.tile([1, P], BF16)     # K=1 lhsT for v-bias matmul
            qT = sb.tile([P, MT, T], BF16)
            kT = sb.tile([P, MT, T], BF16)
            vsb = sb.tile([P, ST, HG, HD + 1], BF16)
            po = sb.tile([P, MT, T], BF16)     # scaled outT, out-proj lhsT
            stash = [sb.tile([P, ST, 1024], BF16, name=f"stash{i}")
                     for i in range(2)]

            # warmup: a 1-column matmul as soon as ones_r is set starts the
            # PE p-state clock, so real matmuls (arriving ~4us later, past
            # the 3us ramp) run at full clock from the first chunk.
            nc.vector.memset(ones_r[:], 1.0)
            warm = ps.tile([1, 1], F32, tag="pv", bufs=4, name="warm")
            nc.tensor.matmul(warm[:], ones_r[:, 0:1], ones_r[:, 0:1],
                             start=True, stop=True)

            # startup DMAs: first hT chunk on the SP queue; q/k weights in
            # k-halves on the ACT HWDGE queue, interleaved so the first QKV
            # matmuls get their operands as early as possible.
            hT_r = hT_d.rearrange("(k p) t -> p k t", p=P)
            wq_r = wqT_d.rearrange("(k p) r -> p k r", p=P)
            wk_r = wkT_d.rearrange("(k p) r -> p k r", p=P)
            nc.sync.dma_start(hT[:, 0:2, 0:512], hT_r[:, 0:2, 0:512])
            nc.scalar.dma_start(wqT[:, 0:4, :], wq_r[:, 0:4, :])
            nc.sync.dma_start(hT[:, 2:4, 0:512], hT_r[:, 2:4, 0:512])
            nc.scalar.dma_start(wqT[:, 4:8, :], wq_r[:, 4:8, :])
            nc.sync.dma_start(hT[:, 4:6, 0:512], hT_r[:, 4:6, 0:512])
            nc.scalar.dma_start(wkT[:, 0:4, :], wk_r[:, 0:4, :])
            nc.sync.dma_start(hT[:, 6:8, 0:512], hT_r[:, 6:8, 0:512])
            nc.scalar.dma_start(wkT[:, 4:8, :], wk_r[:, 4:8, :])
            nc.sync.dma_start(bq[:], bq_d.rearrange("(m p) one -> p (m one)", p=P))
            nc.sync.dma_start(bk[:], bk_d.rearrange("(m p) one -> p (m one)", p=P))
            nc.sync.dma_start(bv[:], bv_d[:])
            nc.vector.memset(vsb[:, :, :, HD : HD + 1], 1.0)

            ft_tiles = {}
            emitted_ft = set()

            def new_ft(tch):
                ft_tiles[tch] = sb.tile([P, ST, 512], BF16,
                                        tag=f"ft{tch % 2}", bufs=1, name=f"ft_t{tch}")

            def ft_need(tch, st):
                """Ensure f tiles for (tch, st..st+2) are loading."""
                if tch not in ft_tiles:
                    new_ft(tch)
                for s in range(st, min(st + 3, ST)):
                    if (tch, s) not in emitted_ft:
                        emitted_ft.add((tch, s))
                        nc.sync.dma_start(
                            ft_tiles[tch][:, s, :],
                            fT_d[s * P : (s + 1) * P,
                                 tch * 512 : (tch + 1) * 512],
                        )

            # ================= virtual-clock scheduler =================
            clk = {"pe": 0.0, "act": 0.0, "dve": 0.0, "pool": 0.0}

            exp_done = {}     # (u, st) -> ACT virtual completion of exp
            fmul_done = {}    # (u, st) -> DVE virtual completion of f-mul
            qk_ready = {}     # (w, n, m) -> ACT completion of bias-add
            v_ready = {}      # st -> DVE completion of v copy-out
            po_ready = {}     # u -> DVE completion of both po scale-muls
            pv_done = {}      # (u, st) -> PE completion of PV step
            sc_queue = []     # ACT completion times of in-flight exps
            e_tiles = {}      # st -> e tile for live u0
            pv_pairs = {}
            norm_hold = []    # (u, h, recip, praw, recip_done)

            def new_pv_pair(u):
                pv_pairs[u] = [ps.tile([HD + 1, 512], F32, tag="pv", bufs=4,
                                       name=f"pv_{u}_{a}") for a in range(2)]

            def emit_chain(w, n, m):
                w_sb, b_sb, o_sb = ((wqT, bq, qT) if w == "q" else (wkT, bk, kT))
                nsl = slice(n * 512, (n + 1) * 512)
                acc = ps.tile([P, 512], F32, tag="pv", bufs=4,
                              name=f"{w}acc_{n}_{m}")
                for k in range(KT):
                    nc.tensor.matmul(
                        acc[:], w_sb[:, k, m * P : (m + 1) * P], hT[:, k, nsl],
                        start=(k == 0), stop=(k == KT - 1),
                    )
                clk["pe"] += KT * MM512
                nc.scalar.activation(o_sb[:, m, nsl], acc[:], AF.Identity,
                                     bias=b_sb[:, m : m + 1])
                clk["act"] = max(clk["act"], clk["pe"]) + BIAS + ACT_OH
                qk_ready[(w, n, m)] = clk["act"]

            def emit_vchain(st):
                acc = ps.tile([P, R], F32, tag="pv", bufs=4, name=f"vacc_{st}")
                for k in range(KT):
                    nc.tensor.matmul(
                        acc[:], hT[:, k, st * P : (st + 1) * P], wvT[:, k, :],
                        start=(k == 0), stop=False,
                    )
                nc.tensor.matmul(acc[:], ones_r[:], bv[:], start=False, stop=True)
                clk["pe"] += (KT + 1) * MM256
                nc.vector.tensor_copy(
                    vsb[:, st, :, 0:HD],
                    acc[:].rearrange("p (h d) -> p h d", h=HG),
                )
                clk["dve"] = max(clk["dve"], clk["pe"]) + VCOPY
                v_ready[st] = clk["dve"]

            def emit_front(u, st):
                tch, j = divmod(u, 2)
                ft_need(tch, st)
                tsl = slice(tch * 512, (tch + 1) * 512)
                ssl = slice(st * P, (st + 1) * P)
                sc = ps.tile([P, 1024], F32, tag="sc", bufs=2, name=f"sc{u}_{st}")
                for a in range(2):
                    rows = slice(a * HD, (a + 1) * HD)
                    nc.tensor.matmul(
                        sc[:, a * 512 : (a + 1) * 512],
                        kT[rows, j, ssl], qT[rows, j, tsl],
                        start=True, stop=True,
                    )
                clk["pe"] = max(clk["pe"], qk_ready[("q", tch, j)],
                                qk_ready[("k", st // 4, j)]) + 2 * MM512
                if u == 0:
                    dst = sb.tile([P, 1024], BF16, tag="e", bufs=E_BUFS,
                                  name=f"e00_{st}")
                    e_tiles[st] = dst
                else:
                    dst = stash[u % 2][:, st, :]
                nc.scalar.activation(dst, sc[:], AF.Exp)
                war = pv_done.get((u - 2, st), 0.0) if u >= 3 else 0.0
                clk["act"] = max(clk["act"], clk["pe"], war) + EXP + ACT_OH
                exp_done[(u, st)] = clk["act"]
                sc_queue.append(clk["act"])
                for a in range(2):
                    half = slice(a * 512, (a + 1) * 512)
                    nc.vector.tensor_mul(dst[:, half], dst[:, half],
                                         ft_tiles[tch][:, st, :])
                clk["dve"] = max(clk["dve"], clk["act"]) + 2 * FMUL
                fmul_done[(u, st)] = clk["dve"]

            def emit_pv(u, st):
                """PV step: u0 from its e tile, others from the stash."""
                tch, j = divmod(u, 2)
                if u == 0:
                    src = e_tiles.pop(st)
                else:
                    src = stash[u % 2][:, st, :]
                for a in range(2):
                    nc.tensor.matmul(
                        pv_pairs[u][a][:], vsb[:, st, 2 * j + a, :],
                        src[:, a * 512 : (a + 1) * 512],
                        start=(st == 0), stop=(st == ST - 1),
                    )
                clk["pe"] = max(clk["pe"], fmul_done[(u, st)], v_ready[st])
                clk["pe"] += 2 * MM512
                pv_done[(u, st)] = clk["pe"]
                if st == ST - 1:
                    emit_norm(u)

            def emit_norm(u):
                """Rowsum reciprocals only; the scale-multiply reads the PSUM
                accumulator directly later, so no praw staging copy at all."""
                tch, j = divmod(u, 2)
                pvp = pv_pairs[u]
                for a in range(2):
                    h = 2 * j + a
                    recip = sb.tile([1, 512], BF16, tag="recip", bufs=4,
                                    name=f"recip_{tch}_{h}")
                    nc.vector.reciprocal(recip[:], pvp[a][HD : HD + 1, :])
                    clk["dve"] = max(clk["dve"], pv_done[(u, ST - 1)]) + RECIP
                    norm_hold.append((u, h, recip, pvp[a], clk["dve"]))

            def flush_norms(force=False):
                while norm_hold:
                    u, h, recip, pva, rdone = norm_hold[0]
                    pool_t = max(clk["pool"], rdone) + BCAST
                    if not force and pool_t > clk["dve"] + 400:
                        break
                    norm_hold.pop(0)
                    clk["pool"] = pool_t
                    tch = u // 2
                    bcs = sb.tile([HD, 512], BF16, tag="bcs", bufs=4,
                                  name=f"bcs_{tch}_{h}")
                    nc.gpsimd.partition_broadcast(bcs[:], recip[:])
                    nc.vector.tensor_mul(
                        po[(h % 2) * HD : (h % 2) * HD + HD, h // 2,
                           tch * 512 : (tch + 1) * 512],
                        pva[0:HD, :], bcs[:],
                    )
                    clk["dve"] = max(clk["dve"], clk["pool"]) + OSB_DVE
                    po_ready[u] = max(po_ready.get(u, 0.0), clk["dve"])

            def emit_burst(i):
                tt, n = divmod(i, 2)
                tch = tt // 4
                fin = ps.tile([P, 512], F32, tag="pv", bufs=4, name=f"fin_{tt}_{n}")
                osb = sb.tile([P, 512], BF16, tag="osb", bufs=3, name=f"osb_{tt}_{n}")
                gate = max(po_ready[2 * tch], po_ready[2 * tch + 1])
                for j in range(MT):
                    nc.tensor.matmul(
                        fin[:], po[:, j, tt * P : (tt + 1) * P],
                        woT[:, j, n * 512 : (n + 1) * 512],
                        start=(j == 0), stop=(j == MT - 1),
                    )
                clk["pe"] = max(clk["pe"], gate) + 2 * MM512
                # once the exp stream is exhausted ACT is truly idle;
                # alternate engines there regardless of the virtual clocks
                use_act = (i % 2 == 0) if fi >= len(front_list) \
                    else clk["act"] <= clk["dve"]
                if use_act:
                    nc.scalar.copy(osb[:], fin[:])
                    clk["act"] = max(clk["act"], clk["pe"]) + OSB_ACT + ACT_OH
                else:
                    nc.vector.tensor_copy(osb[:], fin[:])
                    clk["dve"] = max(clk["dve"], clk["pe"]) + OSB_DVE
                nc.sync.dma_start(
                    out_d[tt * P : (tt + 1) * P, n * 512 : (n + 1) * 512], osb[:]
                )

            # ---------- work lists ----------
            chain_list = []
            for n in range(NCH):
                for m in range(MT):
                    chain_list.append(("q", n, m))
                    chain_list.append(("k", n, m))
                for st in range(4 * n, 4 * n + 4):
                    chain_list.append(("v", st))

            front_list = []
            for n in range(NCH):
                for st in range(4 * n, 4 * n + 4):
                    front_list.append((0, st))
                    front_list.append((1, st))
                if n >= 1:
                    for st in range(4 * (n - 1), 4 * n):
                        front_list.append((2, st))
            for st in range(12, ST):
                front_list.append((2, st))
            for u in range(3, 8):
                for st in range(ST):
                    front_list.append((u, st))

            # PV steps: u0 in P0 (live), then u1..u7 sequentially.
            pv_list = [(0, st) for st in range(ST)]
            pv_list += [(u, st) for u in range(1, 8) for st in range(ST)]
            burst_list = list(range(32))

            ci = fi = bi = gi = 0
            emitted_wo = False

            def front_ok(strict):
                if fi >= len(front_list):
                    return False
                u, st = front_list[fi]
                tch, j = divmod(u, 2)
                if ("q", tch, j) not in qk_ready:
                    return False
                if ("k", st // 4, j) not in qk_ready:
                    return False
                if u >= 3 and (u - 2, st) not in pv_done:
                    return False  # stash WAR: hard correctness dependency
                if not strict:
                    return True
                pend = sum(1 for t in sc_queue[-2:] if t > clk["pe"])
                return pend < 2

            def pv_ok(strict):
                if bi >= len(pv_list):
                    return False
                u, st = pv_list[bi]
                if (u, st) not in fmul_done or st not in v_ready:
                    return False
                if u == 1 and ci < len(chain_list):
                    return False  # PSUM: chains still rotating pv slots
                if u >= 1 and (u - 1 if u > 1 else 0, ST - 1) not in pv_done:
                    return False  # one stash-back pair at a time
                if not strict:
                    return True
                return fmul_done[(u, st)] <= clk["pe"] + 300

            def burst_ok(strict):
                if gi >= len(burst_list) or not emitted_wo:
                    return False
                tch = burst_list[gi] // 8
                if 2 * tch not in po_ready or 2 * tch + 1 not in po_ready:
                    return False
                if not strict:
                    return True
                return max(po_ready[2 * tch], po_ready[2 * tch + 1]) \
                    <= clk["pe"] + 300

            new_pv_pair(0)

            while (ci < len(chain_list) or fi < len(front_list)
                   or bi < len(pv_list) or gi < len(burst_list) or norm_hold):
                flush_norms()

                if clk["act"] - clk["pe"] < LOOKAHEAD and front_ok(True):
                    u, st = front_list[fi]
                    fi += 1
                    emit_front(u, st)
                    continue
                if pv_ok(True):
                    u, st = pv_list[bi]
                    bi += 1
                    if u not in pv_pairs:
                        new_pv_pair(u)
                    emit_pv(u, st)
                    continue
                if ci < len(chain_list):
                    item = chain_list[ci]
                    ci += 1
                    if item[0] == "v":
                        if item[1] == 0:
                            nc.sync.dma_start(
                                wvT[:],
                                wvT_d.rearrange("(k p) r -> p k r", p=P))
                        emit_vchain(item[1])
                    else:
                        w, n, m = item
                        if w == "q" and m == 0 and n + 1 < NCH:
                            psl = slice((n + 1) * 512, (n + 2) * 512)
                            for kk in range(0, KT, 2):
                                nc.sync.dma_start(
                                    hT[:, kk : kk + 2, psl],
                                    hT_r[:, kk : kk + 2, psl])
                        emit_chain(w, n, m)
                    if ci == len(chain_list):
                        nc.sync.dma_start(
                            woT[:], woT_d.rearrange("(m p) d -> p m d", p=P))
                        emitted_wo = True
                    continue
                if burst_ok(True):
                    emit_burst(burst_list[gi])
                    gi += 1
                    continue

                # nothing cleanly ready: emit the least-stalling item
                if pv_ok(False):
                    u, st = pv_list[bi]
                    bi += 1
                    if u not in pv_pairs:
                        new_pv_pair(u)
                    emit_pv(u, st)
                elif burst_ok(False):
                    emit_burst(burst_list[gi])
                    gi += 1
                elif front_ok(False):
                    u, st = front_list[fi]
                    fi += 1
                    emit_front(u, st)
                elif norm_hold:
                    flush_norms(force=True)
                else:
                    raise RuntimeError(
                        f"scheduler wedged: ci={ci} fi={fi} bi={bi} gi={gi}")

    return nc


_NC = None
_LAST_RESULT = None


def _get_nc():
    global _NC
    if _NC is None:
        _NC = build_bass()
        if not _NC.is_finalized():
            _NC.finalize()
    return _NC


def kernel(hidden_states, focused_attention, Wq, bq, Wk, bk, Wv, bv, Wo, bo):
    bf = ml_dtypes.bfloat16
    hT = [np.ascontiguousarray(hidden_states[b].T).astype(bf) for b in range(B)]
    fT = [np.ascontiguousarray(focused_attention[b].T).astype(bf) for b in range(B)]

    in_maps = []
    for c in range(N_CORES):
        b, g = divmod(c, 4)
        rows = slice(g * R, (g + 1) * R)
        in_maps.append({
            "hT": hT[b],
            "fT": fT[b],
            "wqT": np.ascontiguousarray((Wq[rows] * SCALING).T).astype(bf),
            "wkT": np.ascontiguousarray(Wk[rows].T).astype(bf),
            "wvT": np.ascontiguousarray(Wv[rows].T).astype(bf),
            "woT": np.ascontiguousarray(Wo[:, rows].T).astype(bf),
            "bq": np.ascontiguousarray((bq[rows] * SCALING)[:, None]).astype(np.float32),
            "bk": np.ascontiguousarray(bk[rows][:, None]).astype(np.float32),
            "bv": np.ascontiguousarray(bv[rows][None, :]).astype(bf),
        })

    res = run_bass_kernel_spmd(_get_nc(), in_maps, list(range(N_CORES)))
    global _LAST_RESULT
    _LAST_RESULT = res
    out = np.zeros((B, T, D), dtype=np.float32)
    for c in range(N_CORES):
        out[c // 4] += np.asarray(res.results[c]["out_partial"], dtype=np.float32)
    out += np.asarray(bo, dtype=np.float32)[None, None, :]
    return out


# revision 17
# speedup vs baseline: 1.1935x; 1.0353x over previous
"""BartAttention (focused-attention variant) Trainium2 Bass kernel.

Problem (hardcoded): B=2, T=2048, D=1024, H=16 heads, hd=64.
  q = (h @ Wq.T + bq) * hd**-0.5 ; k = h @ Wk.T + bk ; v = h @ Wv.T + bv
  scores = q @ k.T per head ; e = f * exp(scores) ; attn = e / rowsum(e)
  out = (attn @ v) @ Wo.T + bo

Sharding over 8 cores: batch (2) x head-group (4 groups of 4 heads).
Each core computes its heads' QKV, attention, and a partial out-projection
(contraction over its 256 d-columns of Wo); host sums the 4 bf16 partials
per batch in f32 and adds bo.

On-device layout (per core):
  hT   [1024, 2048] bf16   hidden.T               (c on partitions)
  qT,kT [256, 2048] bf16   q.T / k.T              (head*hd on partitions)
  v    [2048, 4, 65] bf16  v per head + ones col  (s on partitions)
  scores computed transposed: sT[s,t] = k @ q.T so that e=f.T*exp(sT) has
  s on partitions, which is the contraction dim of the PV matmul.
  PV: outT_aug[65, t] = [v | 1].T @ e  -> row 64 = rowsum(e) per t (exact fp32).
  out-proj: final[t, m] = outT.T @ Wo_slice.T, scaled per head by 1/rowsum
  (1/rowsum broadcast across partitions by GPSIMD, multiply on DVE at 2x).

The 8 head-pair units (tch, j) split into a "front" (scores -> exp ->
f-mul, needs only the 2 `sc` PSUM tiles; all units but u0 stash the result
in one of two SBUF e-stashes, u0 chains its PV live) and a "back" (16 PV
accumulation steps from the stash, needs only 2 `pv` PSUM banks).

Emission is driven by a virtual-clock list scheduler: per-engine virtual
times (PE/ACT/DVE/Pool) advance as instructions are emitted, a front step
is emitted only when ACT would otherwise run dry AND an `sc` slot will be
free when its scores reach the PE queue head (engine queues are in-order,
so an emitted-but-blocked instruction stalls everything behind it), and
otherwise ACT-independent PE filler is emitted: QKV chains during the
first quarter, then PV back-steps and out-projection bursts.  This keeps
the PE queue free of head-of-line blocking, which is what limited
phase-ordered emission.

Engine placement: PE matmuls ~165us (binding); ACT = exps + q/k bias adds
+ some out-proj staging; DVE = f-muls, v copy, reciprocal, praw copies,
po scale-muls, rest of out-proj staging; Pool = 1/rowsum broadcasts.
"""

import numpy as np
import ml_dtypes

import concourse.bass as bass
import concourse.bacc as bacc
import concourse.mybir as mybir
from concourse.tile import TileContext
from concourse.bass_utils import run_bass_kernel_spmd

BF16 = mybir.dt.bfloat16
F32 = mybir.dt.float32
AF = mybir.ActivationFunctionType

B, T, D = 2, 2048, 1024
H, HD = 16, 64
HG = 4               # heads per core
R = HG * HD          # 256 d-rows per core
SCALING = HD ** -0.5
N_CORES = 8

P = 128
KT = D // P          # 8 k-tiles for QKV contraction
MT = R // P          # 2 m-tiles of qT/kT rows
NCH = T // 512       # 4 chunks of 512 along t
ST = T // P          # 16 s-tiles

E_BUFS = 5

# virtual-clock costs (ns), from the TRN2 cost model
MM512 = 512 * (1e9 / 2.4e9)          # one 512-col bf16 matmul
MM256 = 256 * (1e9 / 2.4e9)
EXP = (1024 + 222) * (1e9 / 1.2e9)   # [128,1024] exp, PSUM->SBUF
BIAS = (512 + 222) * (1e9 / 1.2e9)   # bias-add copy on ACT
FMUL = (256 + 58) * (1e9 / 0.96e9)   # [128,512] bf16 mul at DVE 2x
VCOPY = (256 + 120) * (1e9 / 0.96e9)
PRAW = (512 + 120) * (1e9 / 0.96e9)
RECIP = (512 + 120) * (1e9 / 0.96e9)
POMUL = (256 + 58) * (1e9 / 0.96e9)
OSB_DVE = (512 + 120) * (1e9 / 0.96e9)
OSB_ACT = (512 + 222) * (1e9 / 1.2e9)
BCAST = 512 * (1e9 / 1.2e9) + 95
ACT_OH = 60.0                        # dispatch/seq overheads per ACT instr
LOOKAHEAD = 1500.0                   # emit front when ACT lead < this


def build_bass():
    nc = bacc.Bacc()

    hT_d = nc.declare_dram_parameter("hT", [D, T], BF16, isOutput=False)
    fT_d = nc.declare_dram_parameter("fT", [T, T], BF16, isOutput=False)
    wqT_d = nc.declare_dram_parameter("wqT", [D, R], BF16, isOutput=False)
    wkT_d = nc.declare_dram_parameter("wkT", [D, R], BF16, isOutput=False)
    wvT_d = nc.declare_dram_parameter("wvT", [D, R], BF16, isOutput=False)
    woT_d = nc.declare_dram_parameter("woT", [R, D], BF16, isOutput=False)
    bq_d = nc.declare_dram_parameter("bq", [R, 1], F32, isOutput=False)
    bk_d = nc.declare_dram_parameter("bk", [R, 1], F32, isOutput=False)
    bv_d = nc.declare_dram_parameter("bv", [1, R], BF16, isOutput=False)
    out_d = nc.declare_dram_parameter("out_partial", [T, D], BF16, isOutput=True)

    with TileContext(nc) as tc:
        with (
            nc.allow_low_precision(reason="bf16 pipeline is intentional"),
            tc.tile_pool(name="sb", bufs=1) as sb,
            tc.tile_pool(name="ps", bufs=1, space="PSUM") as ps,
        ):
            # ---- persistent SBUF tensors ----
            hT = sb.tile([P, KT, T], BF16)
            wqT = sb.tile([P, KT, R], BF16)
            wkT = sb.tile([P, KT, R], BF16)
            wvT = sb.tile([P, KT, R], BF16)
            woT = sb.tile([P, MT, D], BF16)
            bq = sb.tile([P, MT], F32)
            bk = sb.tile([P, MT], F32)
            bv = sb.tile([1, R], BF16)
            ones_r = sb.tile([1, P], BF16)     # K=1 lhsT for v-bias matmul
            qT = sb.tile([P, MT, T], BF16)
            kT = sb.tile([P, MT, T], BF16)
            vsb = sb.tile([P, ST, HG, HD + 1], BF16)
            po = sb.tile([P, MT, T], BF16)     # scaled outT, out-proj lhsT
            stash = [sb.tile([P, ST, 1024], BF16, name=f"stash{i}")
                     for i in range(2)]

            # warmup: a 1-column matmul as soon as ones_r is set starts the
            # PE p-state clock, so real matmuls (arriving ~4us later, past
            # the 3us ramp) run at full clock from the first chunk.
            nc.vector.memset(ones_r[:], 1.0)
            warm = ps.tile([1, 1], F32, tag="pv", bufs=4, name="warm")
            nc.tensor.matmul(warm[:], ones_r[:, 0:1], ones_r[:, 0:1],
                             start=True, stop=True)

            # startup DMAs: first hT chunk on the SP queue; q/k weights in
            # k-halves on the ACT HWDGE queue, interleaved so the first QKV
            # matmuls get their operands as early as possible.
            hT_r = hT_d.rearrange("(k p) t -> p k t", p=P)
            wq_r = wqT_d.rearrange("(k p) r -> p k r", p=P)
            wk_r = wkT_d.rearrange("(k p) r -> p k r", p=P)
            nc.sync.dma_start(hT[:, 0:2, 0:512], hT_r[:, 0:2, 0:512])
            nc.scalar.dma_start(wqT[:, 0:4, :], wq_r[:, 0:4, :])
            nc.sync.dma_start(hT[:, 2:4, 0:512], hT_r[:, 2:4, 0:512])
            nc.scalar.dma_start(wqT[:, 4:8, :], wq_r[:, 4:8, :])
            nc.sync.dma_start(hT[:, 4:6, 0:512], hT_r[:, 4:6, 0:512])
            nc.scalar.dma_start(wkT[:, 0:4, :], wk_r[:, 0:4, :])
            nc.sync.dma_start(hT[:, 6:8, 0:512], hT_r[:, 6:8, 0:512])
            nc.scalar.dma_start(wkT[:, 4:8, :], wk_r[:, 4:8, :])
            nc.sync.dma_start(bq[:], bq_d.rearrange("(m p) one -> p (m one)", p=P))
            nc.sync.dma_start(bk[:], bk_d.rearrange("(m p) one -> p (m one)", p=P))
            nc.sync.dma_start(bv[:], bv_d[:])
            nc.vector.memset(vsb[:, :, :, HD : HD + 1], 1.0)

            ft_tiles = {}
            emitted_ft = set()

            def new_ft(tch):
                ft_tiles[tch] = sb.tile([P, ST, 512], BF16,
                                        tag=f"ft{tch % 2}", bufs=1, name=f"ft_t{tch}")

            def ft_need(tch, st):
                """Ensure f tiles for (tch, st..st+2) are loading."""
                if tch not in ft_tiles:
                    new_ft(tch)
                for s in range(st, min(st + 3, ST)):
                    if (tch, s) not in emitted_ft:
                        emitted_ft.add((tch, s))
                        nc.sync.dma_start(
                            ft_tiles[tch][:, s, :],
                            fT_d[s * P : (s + 1) * P,
                                 tch * 512 : (tch + 1) * 512],
                        )

            # ================= virtual-clock scheduler =================
            clk = {"pe": 0.0, "act": 0.0, "dve": 0.0, "pool": 0.0}

            exp_done = {}     # (u, st) -> ACT virtual completion of exp
            fmul_done = {}    # (u, st) -> DVE virtual completion of f-mul
            qk_ready = {}     # (w, n, m) -> ACT completion of bias-add
            v_ready = {}      # st -> DVE completion of v copy-out
            po_ready = {}     # u -> DVE completion of both po scale-muls
            pv_done = {}      # (u, st) -> PE completion of PV step
            sc_queue = []     # ACT completion times of in-flight exps
            e_tiles = {}      # st -> e tile for live u0
            pv_pairs = {}
            norm_hold = []    # (u, h, recip, praw, recip_done)

            def new_pv_pair(u):
                pv_pairs[u] = [ps.tile([HD + 1, 512], F32, tag="pv", bufs=4,
                                       name=f"pv_{u}_{a}") for a in range(2)]

            def emit_chain(w, n, m):
                w_sb, b_sb, o_sb = ((wqT, bq, qT) if w == "q" else (wkT, bk, kT))
                nsl = slice(n * 512, (n + 1) * 512)
                acc = ps.tile([P, 512], F32, tag="pv", bufs=4,
                              name=f"{w}acc_{n}_{m}")
                for k in range(KT):
                    nc.tensor.matmul(
                        acc[:], w_sb[:, k, m * P : (m + 1) * P], hT[:, k, nsl],
                        start=(k == 0), stop=(k == KT - 1),
                    )
                clk["pe"] += KT * MM512
                nc.scalar.activation(o_sb[:, m, nsl], acc[:], AF.Identity,
                                     bias=b_sb[:, m : m + 1])
                clk["act"] = max(clk["act"], clk["pe"]) + BIAS + ACT_OH
                qk_ready[(w, n, m)] = clk["act"]

            def emit_vchain(st):
                acc = ps.tile([P, R], F32, tag="pv", bufs=4, name=f"vacc_{st}")
                for k in range(KT):
                    nc.tensor.matmul(
                        acc[:], hT[:, k, st * P : (st + 1) * P], wvT[:, k, :],
                        start=(k == 0), stop=False,
                    )
                nc.tensor.matmul(acc[:], ones_r[:], bv[:], start=False, stop=True)
                clk["pe"] += (KT + 1) * MM256
                nc.vector.tensor_copy(
                    vsb[:, st, :, 0:HD],
                    acc[:].rearrange("p (h d) -> p h d", h=HG),
                )
                clk["dve"] = max(clk["dve"], clk["pe"]) + VCOPY
                v_ready[st] = clk["dve"]

            def emit_front(u, st):
                tch, j = divmod(u, 2)
                ft_need(tch, st)
                tsl = slice(tch * 512, (tch + 1) * 512)
                ssl = slice(st * P, (st + 1) * P)
                sc = ps.tile([P, 1024], F32, tag="sc", bufs=2, name=f"sc{u}_{st}")
                for a in range(2):
                    rows = slice(a * HD, (a + 1) * HD)
                    nc.tensor.matmul(
                        sc[:, a * 512 : (a + 1) * 512],
                        kT[rows, j, ssl], qT[rows, j, tsl],
                        start=True, stop=True,
                    )
                clk["pe"] = max(clk["pe"], qk_ready[("q", tch, j)],
                                qk_ready[("k", st // 4, j)]) + 2 * MM512
                if u == 0:
                    dst = sb.tile([P, 1024], BF16, tag="e", bufs=E_BUFS,
                                  name=f"e00_{st}")
                    e_tiles[st] = dst
                else:
                    dst = stash[u % 2][:, st, :]
                nc.scalar.activation(dst, sc[:], AF.Exp)
                war = pv_done.get((u - 2, st), 0.0) if u >= 3 else 0.0
                clk["act"] = max(clk["act"], clk["pe"], war) + EXP + ACT_OH
                exp_done[(u, st)] = clk["act"]
                sc_queue.append(clk["act"])
                for a in range(2):
                    half = slice(a * 512, (a + 1) * 512)
                    nc.vector.tensor_mul(dst[:, half], dst[:, half],
                                         ft_tiles[tch][:, st, :])
                clk["dve"] = max(clk["dve"], clk["act"]) + 2 * FMUL
                fmul_done[(u, st)] = clk["dve"]

            def emit_pv(u, st):
                """PV step: u0 from its e tile, others from the stash."""
                tch, j = divmod(u, 2)
                if u == 0:
                    src = e_tiles.pop(st)
                else:
                    src = stash[u % 2][:, st, :]
                for a in range(2):
                    nc.tensor.matmul(
                        pv_pairs[u][a][:], vsb[:, st, 2 * j + a, :],
                        src[:, a * 512 : (a + 1) * 512],
                        start=(st == 0), stop=(st == ST - 1),
                    )
                clk["pe"] = max(clk["pe"], fmul_done[(u, st)], v_ready[st])
                clk["pe"] += 2 * MM512
                pv_done[(u, st)] = clk["pe"]
                if st == ST - 1:
                    emit_norm(u)

            def emit_norm(u):
                """Rowsum reciprocals only; the scale-multiply reads the PSUM
                accumulator directly later, so no praw staging copy at all."""
                tch, j = divmod(u, 2)
                pvp = pv_pairs[u]
                for a in range(2):
                    h = 2 * j + a
                    recip = sb.tile([1, 512], BF16, tag="recip", bufs=4,
                                    name=f"recip_{tch}_{h}")
                    nc.vector.reciprocal(recip[:], pvp[a][HD : HD + 1, :])
                    clk["dve"] = max(clk["dve"], pv_done[(u, ST - 1)]) + RECIP
                    norm_hold.append((u, h, recip, pvp[a], clk["dve"]))

            def flush_norms(force=False):
                while norm_hold:
                    u, h, recip, pva, rdone = norm_hold[0]
                    pool_t = max(clk["pool"], rdone) + BCAST
                    if not force and pool_t > clk["dve"] + 400:
                        break
                    norm_hold.pop(0)
                    clk["pool"] = pool_t
                    tch = u // 2
                    bcs = sb.tile([HD, 512], BF16, tag="bcs", bufs=2,
                                  name=f"bcs_{tch}_{h}")
                    nc.gpsimd.partition_broadcast(bcs[:], recip[:])
                    nc.vector.tensor_mul(
                        po[(h % 2) * HD : (h % 2) * HD + HD, h // 2,
                           tch * 512 : (tch + 1) * 512],
                        pva[0:HD, :], bcs[:],
                    )
                    clk["dve"] = max(clk["dve"], clk["pool"]) + OSB_DVE
                    po_ready[u] = max(po_ready.get(u, 0.0), clk["dve"])

            def emit_burst(tt):
                """Both out-proj halves of row-tile tt, one merged DMA."""
                tch = tt // 4
                osb = sb.tile([P, 1024], BF16, tag="osb", bufs=4, name=f"osb_{tt}")
                gate = max(po_ready[2 * tch], po_ready[2 * tch + 1])
                for n in range(2):
                    fin = ps.tile([P, 512], F32, tag="pv", bufs=4,
                                  name=f"fin_{tt}_{n}")
                    for j in range(MT):
                        nc.tensor.matmul(
                            fin[:], po[:, j, tt * P : (tt + 1) * P],
                            woT[:, j, n * 512 : (n + 1) * 512],
                            start=(j == 0), stop=(j == MT - 1),
                        )
                    clk["pe"] = max(clk["pe"], gate) + 2 * MM512
                    # once the exp stream is exhausted ACT is truly idle;
                    # alternate engines there regardless of virtual clocks
                    use_act = (n == 0) if fi >= len(front_list) \
                        else clk["act"] <= clk["dve"]
                    if use_act:
                        nc.scalar.copy(osb[:, n * 512 : (n + 1) * 512], fin[:])
                        clk["act"] = max(clk["act"], clk["pe"]) + OSB_ACT + ACT_OH
                    else:
                        nc.vector.tensor_copy(osb[:, n * 512 : (n + 1) * 512],
                                              fin[:])
                        clk["dve"] = max(clk["dve"], clk["pe"]) + OSB_DVE
                nc.sync.dma_start(
                    out_d[tt * P : (tt + 1) * P, :], osb[:]
                )

            # ---------- work lists ----------
            chain_list = []
            for n in range(NCH):
                for m in range(MT):
                    chain_list.append(("q", n, m))
                    chain_list.append(("k", n, m))
                for st in range(4 * n, 4 * n + 4):
                    chain_list.append(("v", st))

            front_list = []
            for n in range(NCH):
                for st in range(4 * n, 4 * n + 4):
                    front_list.append((0, st))
                    front_list.append((1, st))
                if n >= 1:
                    for st in range(4 * (n - 1), 4 * n):
                        front_list.append((2, st))
            for st in range(12, ST):
                front_list.append((2, st))
            for u in range(3, 8):
                for st in range(ST):
                    front_list.append((u, st))

            # PV steps: u0 in P0 (live), then u1..u7 sequentially.
            pv_list = [(0, st) for st in range(ST)]
            pv_list += [(u, st) for u in range(1, 8) for st in range(ST)]
            burst_list = list(range(16))  # one item per out row-tile tt

            ci = fi = bi = gi = 0
            emitted_wo = False

            def front_ok(strict):
                if fi >= len(front_list):
                    return False
                u, st = front_list[fi]
                tch, j = divmod(u, 2)
                if ("q", tch, j) not in qk_ready:
                    return False
                if ("k", st // 4, j) not in qk_ready:
                    return False
                if u >= 3 and (u - 2, st) not in pv_done:
                    return False  # stash WAR: hard correctness dependency
                if not strict:
                    return True
                pend = sum(1 for t in sc_queue[-2:] if t > clk["pe"])
                return pend < 2

            def pv_ok(strict):
                if bi >= len(pv_list):
                    return False
                u, st = pv_list[bi]
                if (u, st) not in fmul_done or st not in v_ready:
                    return False
                if u == 1 and ci < len(chain_list):
                    return False  # PSUM: chains still rotating pv slots
                if u >= 1 and (u - 1 if u > 1 else 0, ST - 1) not in pv_done:
                    return False  # one stash-back pair at a time
                if not strict:
                    return True
                return fmul_done[(u, st)] <= clk["pe"] + 300

            def burst_ok(strict):
                if gi >= len(burst_list) or not emitted_wo:
                    return False
                tch = burst_list[gi] // 4
                if 2 * tch not in po_ready or 2 * tch + 1 not in po_ready:
                    return False
                if not strict:
                    return True
                return max(po_ready[2 * tch], po_ready[2 * tch + 1]) \
                    <= clk["pe"] + 300

            new_pv_pair(0)

            while (ci < len(chain_list) or fi < len(front_list)
                   or bi < len(pv_list) or gi < len(burst_list) or norm_hold):
                flush_norms()

                if clk["act"] - clk["pe"] < LOOKAHEAD and front_ok(True):
                    u, st = front_list[fi]
                    fi += 1
                    emit_front(u, st)
                    continue
                if pv_ok(True):
                    u, st = pv_list[bi]
                    bi += 1
                    if u not in pv_pairs:
                        new_pv_pair(u)
                    emit_pv(u, st)
                    continue
                if ci < len(chain_list):
                    item = chain_list[ci]
                    ci += 1
                    if item[0] == "v":
                        if item[1] == 0:
                            nc.sync.dma_start(
                                wvT[:],
                                wvT_d.rearrange("(k p) r -> p k r", p=P))
                        emit_vchain(item[1])
                    else:
                        w, n, m = item
                        if w == "q" and m == 0 and n + 1 < NCH:
                            psl = slice((n + 1) * 512, (n + 2) * 512)
                            for kk in range(0, KT, 2):
                                nc.sync.dma_start(
                                    hT[:, kk : kk + 2, psl],
                                    hT_r[:, kk : kk + 2, psl])
                        emit_chain(w, n, m)
                    if ci == len(chain_list):
                        nc.sync.dma_start(
                            woT[:], woT_d.rearrange("(m p) d -> p m d", p=P))
                        emitted_wo = True
                    continue
                if burst_ok(True):
                    emit_burst(burst_list[gi])
                    gi += 1
                    continue

                # nothing cleanly ready: emit the least-stalling item
                if pv_ok(False):
                    u, st = pv_list[bi]
                    bi += 1
                    if u not in pv_pairs:
                        new_pv_pair(u)
                    emit_pv(u, st)
                elif burst_ok(False):
                    emit_burst(burst_list[gi])
                    gi += 1
                elif front_ok(False):
                    u, st = front_list[fi]
                    fi += 1
                    emit_front(u, st)
                elif norm_hold:
                    flush_norms(force=True)
                else:
                    raise RuntimeError(
                        f"scheduler wedged: ci={ci} fi={fi} bi={bi} gi={gi}")

    return nc


_NC = None
_LAST_RESULT = None


def _get_nc():
    global _NC
    if _NC is None:
        _NC = build_bass()
        if not _NC.is_finalized():
            _NC.finalize()
    return _NC


def kernel(hidden_states, focused_attention, Wq, bq, Wk, bk, Wv, bv, Wo, bo):
    bf = ml_dtypes.bfloat16
    hT = [np.ascontiguousarray(hidden_states[b].T).astype(bf) for b in range(B)]
    fT = [np.ascontiguousarray(focused_attention[b].T).astype(bf) for b in range(B)]

    in_maps = []
    for c in range(N_CORES):
        b, g = divmod(c, 4)
        rows = slice(g * R, (g + 1) * R)
        in_maps.append({
            "hT": hT[b],
            "fT": fT[b],
            "wqT": np.ascontiguousarray((Wq[rows] * SCALING).T).astype(bf),
            "wkT": np.ascontiguousarray(Wk[rows].T).astype(bf),
            "wvT": np.ascontiguousarray(Wv[rows].T).astype(bf),
            "woT": np.ascontiguousarray(Wo[:, rows].T).astype(bf),
            "bq": np.ascontiguousarray((bq[rows] * SCALING)[:, None]).astype(np.float32),
            "bk": np.ascontiguousarray(bk[rows][:, None]).astype(np.float32),
            "bv": np.ascontiguousarray(bv[rows][None, :]).astype(bf),
        })

    res = run_bass_kernel_spmd(_get_nc(), in_maps, list(range(N_CORES)))
    global _LAST_RESULT
    _LAST_RESULT = res
    out = np.zeros((B, T, D), dtype=np.float32)
    for c in range(N_CORES):
        out[c // 4] += np.asarray(res.results[c]["out_partial"], dtype=np.float32)
    out += np.asarray(bo, dtype=np.float32)[None, None, :]
    return out
